# revision 75
# baseline (speedup 1.0000x reference)
"""BehaviorAwareGCNLayer on 8 Trainium2 NeuronCores.

Math (reference):
    hx  = x @ W
    out[r] = (1/deg[r]) * sum_{e: row[e]=r} sim_w[e]*sigmoid(rep[row]+rep[col])*ns[col] * hx[col]
    out += sigmoid(rep) * (x @ W_self);  leaky_relu(out, 0.01)

Device strategy (destination sharding, no collectives):
  - By linearity, W is applied AFTER aggregation: agg[r] = sum coef_e * x[col_e],
    out[r] = (agg[r]/deg[r]) @ W + sigmoid(rep_r)*(x_r @ W_self).
  - Host does LAYOUT only (grouping/padding/fancy-index staging, same as the
    per-edge rep[row]/rep[col]/ns[col] arrays): it also stages the per-edge
    x[col] rows into slot order, so the device reads fully sequential
    streams instead of per-row gathers (dma_gather descriptor generation on
    GPSIMD was the original bottleneck: 2.5ms of Q7 busy time).
  - Core c owns destination rows [c*12500, (c+1)*12500). Edges are grouped
    into chunk-aligned runs by (core, 64-row half-block); run capacities are
    uniform across cores (max, rounded to 128) -> single SPMD program.
  - Slot (chunk ci, partition p) holds one edge. Per-batch tensors are
    chunk-interleaved ([128, NB/ILV, d-or-j, ILV]) so every DVE op has
    contiguous innermost APs on all operands -> 2x_1P perf mode (broadcasts
    ride outer dims), while PE operand slices keep a small ILV*2-byte
    stride (64B+ strides halved the LDWEIGHTS/matmul cadence).
    Per batch of NB chunks:
      * SWDGE DMA streams staged fp8-e4m3 x[col] rows, upconverting to
        bf16 in the DMA datapath (halves the dominant HBM stream)
      * msg[e, :, 0:64, g] = coef * x_col (bf16), row 64 = 1 (for deg)
      * one-hot oh[e, :, j, g] = (row_off == j), j in [0, 64)
      * per chunk, one PE matmul accumulates into the owning pair's PSUM:
        psum[half*64 + j, 0:65] += sum_e oh[e, j] * msg[e, :]
  - coef = sw * sigmoid(rep_row + rep_col) * ns_col is precomputed for ALL
    chunks in 4 bulk instructions at program start.
  - Per 128-row pair (two half-block runs share one [128, 65] PSUM tile):
    one ACT copy drains PSUM into a resident accumulator; every 14 pairs a
    grouped finalize does bulk 1/(deg+eps), sigmoid(rep), cat assembly, then
    per pair: PE transpose + one matmul with [W; W_self], ACT leaky-relu
    into a resident output tile; one bulk DMA out at the end.
"""
import sys

if "/opt/trn_rl_repo" not in sys.path:
    sys.path.insert(0, "/opt/trn_rl_repo")

import numpy as np

P = 128
D = 64
HALF = 64                              # one-hot width / sub-block rows
QPP = P // HALF                        # sub-blocks per 128-row output block
N_NODES = 100000
N_CORES = 8
N_LOC = N_NODES // N_CORES             # 12500 destination rows per core
N_PAIR = (N_LOC + P - 1) // P          # 98 output blocks
N_HB = N_PAIR * QPP                    # sub-blocks incl. trailing virtual pad
LAST_VALID = N_LOC - (N_PAIR - 1) * P  # 84 valid rows in last block
NB = 64                                # chunks per batch
ILV = 4                                # chunk interleave: PE operand stride
NBG = NB // ILV                        #   becomes ILV*2 bytes (4B at ILV=2)
GRP = 14                               # pairs per grouped finalize
# group boundaries: 14-pair groups, tail split finer to shorten the drain
_BOUNDS = [0, 14, 28, 42, 56, 70, 84, 88, 91, 93, 95, 96, 97, 98]
GROUP_ENDS = {_BOUNDS[i + 1]: (_BOUNDS[i], _BOUNDS[i + 1] - _BOUNDS[i])
              for i in range(len(_BOUNDS) - 1)}
DUMMY_OFF = 1000.0                     # one-hot-killing row offset for pads


def _layout(hcap):
    """Chunk-aligned run layout from per-half-block capacities (hcap[hb] is
    a multiple of P edges, shared across cores)."""
    run_start = [0] * N_HB             # slot index where hb's run begins
    chunk_meta = []                    # per chunk: (hb, is_start, is_stop)
    pos = 0
    for hb in range(N_HB):
        run_start[hb] = pos
        nch = int(hcap[hb]) // P
        for k in range(nch):
            chunk_meta.append((hb, k == 0, k == nch - 1))
        pos += int(hcap[hb])
    return run_start, chunk_meta, pos // P


def _build_program(hcap):
    """Emit + compile the single-core SPMD program."""
    import concourse.bacc as bacc
    import concourse.mybir as mybir
    import concourse.tile as tile
    from concourse.masks import make_identity

    f32 = mybir.dt.float32
    bf16 = mybir.dt.bfloat16
    f8 = mybir.dt.float8e4

    _, chunk_meta, C = _layout(hcap)

    nc = bacc.Bacc("TRN2", target_bir_lowering=False, debug=False)

    HEADB = 3          # leading batches staged bf16 (HWDGE, no Q7 wait)
    xg_d = nc.dram_tensor("xg", [P, C * D], f8, kind="ExternalInput")
    xgh_d = nc.dram_tensor("xg_head", [P, HEADB * NB * D], bf16,
                           kind="ExternalInput")
    # packed per-chunk metadata: [rowoff, reprow, repc, sw, nsc] along dim 1
    meta_d = nc.dram_tensor("meta5", [P, 5 * C], bf16, kind="ExternalInput")
    repsh_d = nc.dram_tensor("rep_sh", [P, N_PAIR], f32, kind="ExternalInput")
    xself_d = nc.dram_tensor("x_selfT", [P, N_PAIR * D], bf16,
                             kind="ExternalInput")
    iotam_d = nc.dram_tensor("iota_m", [P, NB * HALF], bf16,
                             kind="ExternalInput")
    wcat_d = nc.dram_tensor("w_cat", [2 * D, D], bf16, kind="ExternalInput")
    out_d = nc.dram_tensor("out", [P, N_PAIR * D], f32, kind="ExternalOutput")

    AL = mybir.AluOpType
    ACT = mybir.ActivationFunctionType

    with tile.TileContext(nc) as tc:
        with (
            tc.tile_pool(name="meta", bufs=1) as meta,
            tc.tile_pool(name="gather", bufs=3) as gpool,
            tc.tile_pool(name="onehot", bufs=3) as opool,
            tc.tile_pool(name="const", bufs=1) as cpool,
            tc.tile_pool(name="fin", bufs=3) as fpool,
            tc.tile_pool(name="psum", bufs=4, space="PSUM") as psum,
            tc.tile_pool(name="psumT", bufs=2, space="PSUM") as psumT,
        ):
            meta_s = meta.tile([P, 5, C], bf16)
            rowoff_s = meta_s[:, 0, :]
            reprow_s = meta_s[:, 1, :]
            repc_s = meta_s[:, 2, :]
            sw_s = meta_s[:, 3, :]
            nsc_s = meta_s[:, 4, :]
            coefb = meta.tile([P, C], bf16)
            repsh_s = meta.tile([P, N_PAIR], f32)
            srep_all = meta.tile([P, N_PAIR], f32)
            xselfb = meta.tile([P, N_PAIR, D], bf16)
            cat_all = meta.tile([P, N_PAIR, 2 * D], bf16)
            acc_all = meta.tile([P, N_PAIR, D + 1], f32)
            outs = meta.tile([P, N_PAIR, D], f32)
            wcat_s = cpool.tile([2 * D, D], bf16)
            ident = cpool.tile([P, P], bf16)
            iotaM = cpool.tile([P, NBG, HALF, ILV], bf16)
            # msg tiles are persistent (not pooled) so their deg-ones row is
            # written once in the prologue instead of every batch
            msg_bufs = [meta.tile([P, NBG, D + 1, ILV], bf16,
                                  name=f"msgbuf{k}")
                        for k in range(3)]
            # prepay the Q7 SWDGE ucode IRAM load (~6us) before batch 0's
            # cast-DMA needs it, overlapped with the prologue loads
            swdge_warm = cpool.tile([P, D], bf16)
            nc.gpsimd.dma_start(
                out=swdge_warm[:].rearrange("p d -> p d"),
                in_=xg_d[:, 0:D])
            # all prologue loads go on the scalar-engine HWDGE queue so the
            # sync queue carries nothing but the xg edge stream. The first
            # HEAD chunks of every per-chunk array load first (small, fixed-
            # cost DMAs) so the first batches start ~20us earlier; the bulk
            # tails follow.
            HEAD = 4 * NB
            meta_dv = meta_d[:].rearrange("p (k c) -> p k c", k=5)
            nc.scalar.dma_start(out=meta_s[:, :, :HEAD],
                                in_=meta_dv[:, :, :HEAD])
            nc.scalar.dma_start(
                out=iotaM[:].rearrange("p b j g -> p (b j g)"),
                in_=iotam_d[:])
            nc.scalar.dma_start(out=meta_s[:, :, HEAD:],
                                in_=meta_dv[:, :, HEAD:])

            make_identity(nc, ident[:])
            for mb in msg_bufs:
                nc.vector.memset(mb[:, :, D:D + 1, :], 1.0)

            # keep the PE clock gate (HAM) warm through the prologue
            warm_ps = psum.tile([P, D + 1], f32, tag="agg", name="warm_ps")
            for _ in range(40):
                nc.tensor.matmul(out=warm_ps[0:HALF, :],
                                 lhsT=ident[:, 0:HALF],
                                 rhs=ident[:, 0:D + 1],
                                 start=True, stop=True)

            # coef = sw * sigmoid(rep_row + rep_col) * ns_col. The head
            # slice is computed in the prologue; the tail pass is emitted
            # mid-loop (see below) so it does not block batches 0-2 in the
            # DVE instruction stream.
            def coef_pass(lo, hi):
                nc.vector.tensor_tensor(out=coefb[:, lo:hi],
                                        in0=reprow_s[:, lo:hi],
                                        in1=repc_s[:, lo:hi], op=AL.add)
                nc.scalar.activation(coefb[:, lo:hi], coefb[:, lo:hi],
                                     ACT.Sigmoid)
                nc.vector.tensor_tensor(out=coefb[:, lo:hi],
                                        in0=coefb[:, lo:hi],
                                        in1=sw_s[:, lo:hi], op=AL.mult)
                nc.vector.tensor_tensor(out=coefb[:, lo:hi],
                                        in0=coefb[:, lo:hi],
                                        in1=nsc_s[:, lo:hi], op=AL.mult)

            coef_pass(0, HEAD)

            # finalize-only inputs
            nc.scalar.dma_start(out=repsh_s[:], in_=repsh_d[:])
            nc.scalar.dma_start(out=xselfb[:].rearrange("p b d -> p (b d)"),
                                in_=xself_d[:])
            nc.scalar.dma_start(out=wcat_s[:], in_=wcat_d[:])

            # the self-term half of cat does not depend on aggregation:
            # compute it once in the prologue (DVE is otherwise idle here)
            nc.scalar.activation(srep_all[:], repsh_s[:], ACT.Sigmoid)
            nc.vector.tensor_tensor(
                out=cat_all[:, :, D:2 * D], in0=xselfb[:],
                in1=srep_all[:].rearrange("p (b o) -> p b o", o=1)
                    .to_broadcast([P, N_PAIR, D]),
                op=AL.mult)

            def finalize_group(lo, n, drain=False):
                dg = fpool.tile([P, GRP], f32, tag="dg")
                nc.any.tensor_scalar_add(
                    out=dg[:, :n],
                    in0=acc_all[:, lo:lo + n, D:D + 1]
                        .rearrange("p b o -> p (b o)"),
                    scalar1=1e-6)
                nc.vector.reciprocal(out=dg[:, :n], in_=dg[:, :n])
                nc.vector.tensor_tensor(
                    out=cat_all[:, lo:lo + n, 0:D],
                    in0=acc_all[:, lo:lo + n, 0:D],
                    in1=dg[:, :n].rearrange("p (b o) -> p b o", o=1)
                        .to_broadcast([P, n, D]),
                    op=AL.mult)
                for k in range(n):
                    pair = lo + k
                    ctp = psumT.tile([P, P], bf16, tag="ctp")
                    nc.tensor.transpose(out=ctp[:], in_=cat_all[:, pair, :],
                                        identity=ident[:])
                    catT = fpool.tile([P, P], bf16, tag="catT")
                    if drain:
                        # in the drain the DVE is idle while ACT serializes
                        nc.vector.tensor_copy(out=catT[:], in_=ctp[:])
                    else:
                        nc.scalar.copy(catT[:], ctp[:])
                    out_ps = psumT.tile([P, D], f32, tag="out_ps")
                    nc.tensor.matmul(out=out_ps[:], lhsT=catT[:],
                                     rhs=wcat_s[:], start=True, stop=True)
                    nc.scalar.activation(outs[:, pair, :], out_ps[:],
                                         ACT.Lrelu, alpha=0.01)
                nc.sync.dma_start(
                    out=out_d[:, lo * D:(lo + n) * D],
                    in_=outs[:, lo:lo + n, :]
                        .rearrange("p b d -> p (b d)"))

            psum_cur = [None]
            pending = []   # finalize groups deferred to the next batch
            for bi, c0 in enumerate(range(0, C, NB)):
                xgb = gpool.tile([P, NBG, D, ILV], bf16, tag="xg")
                if bi < HEADB:
                    # head batches: plain HWDGE load of bf16-staged data, so
                    # the stream starts before the Q7 SWDGE ucode is loaded
                    nc.sync.dma_start(
                        out=xgb[:].rearrange("p b d g -> p (b d g)"),
                        in_=xgh_d[:, c0 * D:(c0 + NB) * D])
                else:
                    # fp8 in HBM, upconverted to bf16 in the SWDGE datapath
                    nc.gpsimd.dma_start(
                        out=xgb[:].rearrange("p b d g -> p (b d g)"),
                        in_=xg_d[:, c0 * D:(c0 + NB) * D])

                msg = msg_bufs[bi % 3]
                nc.vector.tensor_tensor(
                    out=msg[:, :, 0:D, :], in0=xgb[:],
                    in1=coefb[:, c0:c0 + NB]
                        .rearrange("p (b o g) -> p b o g", o=1, g=ILV)
                        .to_broadcast([P, NBG, D, ILV]),
                    op=AL.mult)

                oh = opool.tile([P, NBG, HALF, ILV], bf16, tag="oh")
                nc.vector.tensor_tensor(
                    out=oh[:],
                    in0=rowoff_s[:, c0:c0 + NB]
                        .rearrange("p (b o g) -> p b o g", o=1, g=ILV)
                        .to_broadcast([P, NBG, HALF, ILV]),
                    in1=iotaM[:],
                    op=AL.is_equal)

                if bi == 2:
                    # coef for batches 4+, emitted once batches 0-2 are in
                    # flight (their ops precede it in the DVE stream)
                    coef_pass(HEAD, C)

                # emit deferred finalize groups AFTER this batch's DVE prep:
                # their DVE/PE ops depend on earlier batches' matmuls, so
                # emitting them first would stall the DVE stream and starve
                # the PE of the next batch's one-hot/msg
                for lo, n in pending:
                    finalize_group(lo, n)
                pending = []

                for i in range(NB):
                    hb, is_start, is_stop = chunk_meta[c0 + i]
                    q = hb % QPP
                    if is_start and q == 0:
                        psum_cur[0] = psum.tile([P, D + 1], f32, tag="agg",
                                                name="agg_ps")
                    ps = psum_cur[0]
                    nc.tensor.matmul(
                        out=ps[q * HALF:(q + 1) * HALF, :],
                        lhsT=oh[:, i // ILV, :, i % ILV],
                        rhs=msg[:, i // ILV, 0:D + 1, i % ILV],
                        start=is_start, stop=is_stop,
                        tile_position=(0, q * HALF))
                    if is_stop and q == QPP - 1:
                        pair = hb // QPP
                        if c0 >= C - 2 * NB:
                            # drain region: ACT is the serializer, DVE idles
                            nc.vector.tensor_copy(out=acc_all[:, pair, :],
                                                  in_=ps[:])
                        else:
                            nc.scalar.copy(acc_all[:, pair, :], ps[:])
                        if pair + 1 in GROUP_ENDS:
                            # near the stream's end there is no later batch
                            # prep to protect; finalize eagerly to shorten
                            # the drain
                            if c0 >= C - 3 * NB:
                                finalize_group(*GROUP_ENDS[pair + 1],
                                               drain=True)
                            else:
                                pending.append(GROUP_ENDS[pair + 1])
            for lo, n in pending:
                finalize_group(lo, n, drain=True)

    nc.compile()
    return nc


def _preprocess(x, edge_index, sim_weight, rep, node_signal):
    """Host-side layout: group edges into (core, 64-row half-block) runs,
    pad to uniform chunk-aligned capacities, stage per-edge per-slot arrays
    (including the x[col] rows) in stream order."""
    import ml_dtypes

    bf = ml_dtypes.bfloat16
    row = np.ascontiguousarray(edge_index[0]).astype(np.int64)
    col = np.ascontiguousarray(edge_index[1]).astype(np.int64)
    sw = np.ascontiguousarray(sim_weight).astype(np.float32)
    rep_f = np.ascontiguousarray(rep).astype(np.float32)
    ns_f = np.ascontiguousarray(node_signal).astype(np.float32)
    x_f = np.ascontiguousarray(x).astype(np.float32)
    E = row.shape[0]

    core = row // N_LOC
    lrow = row - core * N_LOC

    # Balanced assignment of local rows to (half-block, offset): deal the
    # degree-sorted rows serpentine-wise across the 196 half-blocks (which
    # nearly equalizes per-run edge counts), then greedily swap rows between
    # bins until every bin is <= 1024 edges. hcap then rounds to 1024 for
    # almost every run instead of 1152 (~10% less chunk padding). The row
    # permutation is inverted when unsharding the output.
    n_real = (N_LOC + HALF - 1) // HALF
    CAP1 = 8 * P      # 1024
    deg_local = np.zeros((N_CORES, N_LOC), dtype=np.int64)
    np.add.at(deg_local, (core, lrow), 1)
    hb_map = np.empty((N_CORES, N_LOC), dtype=np.int64)
    off_map = np.empty((N_CORES, N_LOC), dtype=np.int64)
    for c in range(N_CORES):
        degs = deg_local[c]
        order_r = np.argsort(-degs, kind="stable")
        bins = [[] for _ in range(n_real)]
        sums = np.zeros(n_real, dtype=np.int64)
        for rnd in range((N_LOC + n_real - 1) // n_real):
            chunk = order_r[rnd * n_real:(rnd + 1) * n_real]
            order_b = range(n_real) if rnd % 2 == 0 else \
                range(n_real - 1, -1, -1)
            for k, b in enumerate(order_b):
                if k < len(chunk):
                    bins[b].append(int(chunk[k]))
                    sums[b] += int(degs[chunk[k]])
        # repair: swap rows out of over-cap bins into emptier bins until
        # every bin fits under CAP1
        for _ in range(20000):
            b = int(np.argmax(sums))
            if sums[b] <= CAP1:
                break
            ts = [int(t) for t in np.argsort(sums)[:24] if int(t) != b]
            us = sorted(range(len(bins[b])),
                        key=lambda i: -degs[bins[b][i]])[:12]
            done = False
            for t in ts:
                headroom = int(CAP1 - sums[t])
                for u in us:
                    du = int(degs[bins[b][u]])
                    best, best_dv = None, -1
                    for i, rv in enumerate(bins[t]):
                        dv = int(degs[rv])
                        if dv < du and du - dv <= headroom and dv > best_dv:
                            best, best_dv = i, dv
                    if best is not None:
                        ru, rv = bins[b][u], bins[t][best]
                        bins[b][u], bins[t][best] = rv, ru
                        sums[b] += best_dv - du
                        sums[t] += du - best_dv
                        done = True
                        break
                if done:
                    break
            if not done:
                break
        for b in range(n_real):
            for n, r in enumerate(bins[b]):
                hb_map[c, r] = b
                off_map[c, r] = n
    slot_row = hb_map * HALF + off_map          # [N_CORES, N_LOC]

    hb = hb_map[core, lrow]
    off = off_map[core, lrow].astype(np.float32)

    counts = np.zeros((N_CORES, N_HB), dtype=np.int64)
    np.add.at(counts, (core, hb), 1)
    maxc = counts.max(axis=0)
    assert maxc[:n_real].min() > 0, "empty sub-block run not supported"
    # virtual trailing sub-blocks (rows beyond N_LOC) still get one chunk of
    # pure padding so every pair's PSUM partitions are written
    hcap = np.maximum((-(-maxc // P) * P), P).astype(np.int64)
    # pad the last run so C is a multiple of NB (uniform full batches)
    c_raw = int(hcap.sum()) // P
    hcap[-1] += (-c_raw % NB) * P

    run_start_l, _, C = _layout(hcap)
    assert C % NB == 0
    run_start = np.array(run_start_l, dtype=np.int64)
    total = C * P

    key = core * N_HB + hb
    order = np.argsort(key, kind="stable")
    gcounts = np.bincount(key, minlength=N_CORES * N_HB)
    group_start = np.zeros(N_CORES * N_HB + 1, dtype=np.int64)
    np.cumsum(gcounts, out=group_start[1:])
    rank = np.arange(E, dtype=np.int64) - group_start[key[order]]
    ko = key[order]
    core_o = ko // N_HB
    hb_o = ko % N_HB
    gidx = core_o * total + run_start[hb_o] + rank

    tot = N_CORES * total
    rowoff_p = np.full(tot, DUMMY_OFF, dtype=np.float32)
    sw_p = np.zeros(tot, dtype=np.float32)
    reprow_p = np.zeros(tot, dtype=np.float32)
    repc_p = np.zeros(tot, dtype=np.float32)
    nsc_p = np.zeros(tot, dtype=np.float32)
    rowoff_p[gidx] = off[order]
    sw_p[gidx] = sw[order]
    reprow_p[gidx] = rep_f[row[order]]
    repc_p[gidx] = rep_f[col[order]]
    nsc_p[gidx] = ns_f[col[order]]
    xg = np.zeros((tot, D), dtype=np.float32)
    xg[gidx] = x_f[col[order]]

    def per_core(a):
        return np.ascontiguousarray(
            a.reshape(N_CORES, C, P).transpose(0, 2, 1).astype(bf))

    rowoff_t = per_core(rowoff_p)
    sw_t = per_core(sw_p)
    reprow_t = per_core(reprow_p)
    repc_t = per_core(repc_p)
    nsc_t = per_core(nsc_p)

    # xg stream: per batch of NB chunks, [128, NBG, D, ILV] interleaved so
    # the per-chunk PE operand stride is ILV elements
    xg16 = xg.astype(ml_dtypes.float8_e4m3).reshape(N_CORES, C, P, D)
    xgd = np.empty((N_CORES, P, C * D), dtype=ml_dtypes.float8_e4m3)
    for c0 in range(0, C, NB):
        blk = xg16[:, c0:c0 + NB].reshape(N_CORES, NBG, ILV, P, D)
        blk = blk.transpose(0, 3, 1, 4, 2)     # [8, 128, NBG, D, ILV]
        xgd[:, :, c0 * D:(c0 + NB) * D] = blk.reshape(N_CORES, P, NB * D)

    rep_pad = np.zeros((N_CORES, N_PAIR * P), dtype=np.float32)
    xs_pad = np.zeros((N_CORES, N_PAIR * P, D), dtype=np.float32)
    for c in range(N_CORES):
        rep_pad[c, slot_row[c]] = rep_f[c * N_LOC:(c + 1) * N_LOC]
        xs_pad[c, slot_row[c]] = x_f[c * N_LOC:(c + 1) * N_LOC]
    rep_sh = np.ascontiguousarray(
        rep_pad.reshape(N_CORES, N_PAIR, P).transpose(0, 2, 1))
    x_selfT = np.ascontiguousarray(
        xs_pad.reshape(N_CORES, N_PAIR, P, D).transpose(0, 2, 1, 3)
        .reshape(N_CORES, P, N_PAIR * D).astype(bf))

    iota_m = np.ascontiguousarray(
        np.broadcast_to(
            np.arange(HALF, dtype=np.float32)[None, None, :, None],
            (P, NBG, HALF, ILV)).reshape(P, NB * HALF).astype(bf))

    xg_head = np.ascontiguousarray(xgd[:, :, :3 * NB * D].astype(bf))

    meta5 = np.ascontiguousarray(
        np.stack([rowoff_t, reprow_t, repc_t, sw_t, nsc_t], axis=2)
        .reshape(N_CORES, P, 5 * C))

    return (hcap, xgd, xg_head, meta5, rep_sh, x_selfT, iota_m, slot_row)


_compiled = {}


def _get_program(hcap):
    key = tuple(hcap.tolist())
    if key not in _compiled:
        _compiled[key] = _build_program(hcap)
    return _compiled[key]


def run(x, edge_index, sim_weight, rep, node_signal, W, W_self, trace=False):
    import ml_dtypes
    from concourse.bass_utils import run_bass_kernel_spmd

    (hcap, xgd, xg_head, meta5, rep_sh, x_selfT, iota_m,
     slot_row) = _preprocess(x, edge_index, sim_weight, rep, node_signal)
    w_cat = np.ascontiguousarray(
        np.concatenate([np.asarray(W, dtype=np.float32),
                        np.asarray(W_self, dtype=np.float32)],
                       axis=0).astype(ml_dtypes.bfloat16))
    nc = _get_program(hcap)
    in_maps = []
    for c in range(N_CORES):
        in_maps.append({
            "xg": xgd[c],
            "xg_head": xg_head[c],
            "meta5": meta5[c],
            "rep_sh": rep_sh[c],
            "x_selfT": x_selfT[c],
            "iota_m": iota_m,
            "w_cat": w_cat,
        })
    res = run_bass_kernel_spmd(nc, in_maps, core_ids=list(range(N_CORES)),
                               trace=trace)
    parts = []
    for c in range(N_CORES):
        o = res.results[c]["out"].reshape(P, N_PAIR, D).transpose(1, 0, 2)
        parts.append(o.reshape(N_PAIR * P, D)[slot_row[c]])
    out = np.concatenate(parts, axis=0)
    return out, res


def kernel(x, edge_index, sim_weight, rep, node_signal, W, W_self):
    out, _ = run(x, edge_index, sim_weight, rep, node_signal, W, W_self)
    return out


# revision 76
# speedup vs baseline: 1.1478x; 1.1478x over previous
"""BehaviorAwareGCNLayer on 8 Trainium2 NeuronCores.

Math (reference):
    hx  = x @ W
    out[r] = (1/deg[r]) * sum_{e: row[e]=r} sim_w[e]*sigmoid(rep[row]+rep[col])*ns[col] * hx[col]
    out += sigmoid(rep) * (x @ W_self);  leaky_relu(out, 0.01)

Device strategy (destination sharding, no collectives):
  - By linearity, W is applied AFTER aggregation: agg[r] = sum coef_e * x[col_e],
    out[r] = (agg[r]/deg[r]) @ W + sigmoid(rep_r)*(x_r @ W_self).
  - Host does LAYOUT only (grouping/padding/fancy-index staging, same as the
    per-edge rep[row]/rep[col]/ns[col] arrays): it also stages the per-edge
    x[col] rows into slot order, so the device reads fully sequential
    streams instead of per-row gathers (dma_gather descriptor generation on
    GPSIMD was the original bottleneck: 2.5ms of Q7 busy time).
  - Core c owns destination rows [c*12500, (c+1)*12500). Edges are grouped
    into chunk-aligned runs by (core, 64-row half-block); run capacities are
    uniform across cores (max, rounded to 128) -> single SPMD program.
  - Slot (chunk ci, partition p) holds one edge. Per-batch tensors are
    chunk-interleaved ([128, NB/ILV, d-or-j, ILV]) so every DVE op has
    contiguous innermost APs on all operands -> 2x_1P perf mode (broadcasts
    ride outer dims), while PE operand slices keep a small ILV*2-byte
    stride (64B+ strides halved the LDWEIGHTS/matmul cadence).
    Per batch of NB chunks:
      * SWDGE DMA streams staged fp8-e4m3 x[col] rows, upconverting to
        bf16 in the DMA datapath (halves the dominant HBM stream)
      * msg[e, :, 0:64, g] = coef * x_col (bf16), row 64 = 1 (for deg)
      * one-hot oh[e, :, j, g] = (row_off == j), j in [0, 64)
      * per chunk, one PE matmul accumulates into the owning pair's PSUM:
        psum[half*64 + j, 0:65] += sum_e oh[e, j] * msg[e, :]
  - coef = sw * sigmoid(rep_row + rep_col) * ns_col is precomputed for ALL
    chunks in 4 bulk instructions at program start.
  - Per 128-row pair (two half-block runs share one [128, 65] PSUM tile):
    one ACT copy drains PSUM into a resident accumulator; every 14 pairs a
    grouped finalize does bulk 1/(deg+eps), sigmoid(rep), cat assembly, then
    per pair: PE transpose + one matmul with [W; W_self], ACT leaky-relu
    into a resident output tile; one bulk DMA out at the end.
"""
import sys

if "/opt/trn_rl_repo" not in sys.path:
    sys.path.insert(0, "/opt/trn_rl_repo")

import numpy as np

P = 128
D = 64
HALF = 64                              # one-hot width / sub-block rows
QPP = P // HALF                        # sub-blocks per 128-row output block
N_NODES = 100000
N_CORES = 8
N_LOC = N_NODES // N_CORES             # 12500 destination rows per core
N_PAIR = (N_LOC + P - 1) // P          # 98 output blocks
N_HB = N_PAIR * QPP                    # sub-blocks incl. trailing virtual pad
LAST_VALID = N_LOC - (N_PAIR - 1) * P  # 84 valid rows in last block
NB = 64                                # chunks per batch
ILV = 4                                # chunk interleave: PE operand stride
NBG = NB // ILV                        #   becomes ILV*2 bytes (4B at ILV=2)
GRP = 14                               # pairs per grouped finalize
# group boundaries: 14-pair groups, tail split finer to shorten the drain
_BOUNDS = [0, 14, 28, 42, 56, 70, 84, 88, 91, 93, 95, 96, 97, 98]
GROUP_ENDS = {_BOUNDS[i + 1]: (_BOUNDS[i], _BOUNDS[i + 1] - _BOUNDS[i])
              for i in range(len(_BOUNDS) - 1)}
DUMMY_OFF = 1000.0                     # one-hot-killing row offset for pads


def _layout(hcap):
    """Chunk-aligned run layout from per-half-block capacities (hcap[hb] is
    a multiple of P edges, shared across cores)."""
    run_start = [0] * N_HB             # slot index where hb's run begins
    chunk_meta = []                    # per chunk: (hb, is_start, is_stop)
    pos = 0
    for hb in range(N_HB):
        run_start[hb] = pos
        nch = int(hcap[hb]) // P
        for k in range(nch):
            chunk_meta.append((hb, k == 0, k == nch - 1))
        pos += int(hcap[hb])
    return run_start, chunk_meta, pos // P


def _build_program(hcap):
    """Emit + compile the single-core SPMD program."""
    import concourse.bacc as bacc
    import concourse.mybir as mybir
    import concourse.tile as tile
    from concourse.masks import make_identity

    f32 = mybir.dt.float32
    bf16 = mybir.dt.bfloat16
    f8 = mybir.dt.float8e4

    _, chunk_meta, C = _layout(hcap)

    nc = bacc.Bacc("TRN2", target_bir_lowering=False, debug=False)

    HEADB = 3          # leading batches staged bf16 (HWDGE, no Q7 wait)
    xg_d = nc.dram_tensor("xg", [P, C * D], f8, kind="ExternalInput")
    xgh_d = nc.dram_tensor("xg_head", [P, HEADB * NB * D], bf16,
                           kind="ExternalInput")
    # packed per-chunk metadata: [rowoff, reprow, repc, sw, nsc] along dim 1
    meta_d = nc.dram_tensor("meta5", [P, 5 * C], bf16, kind="ExternalInput")
    repsh_d = nc.dram_tensor("rep_sh", [P, N_PAIR], f32, kind="ExternalInput")
    xself_d = nc.dram_tensor("x_selfT", [P, N_PAIR * D], bf16,
                             kind="ExternalInput")
    iotam_d = nc.dram_tensor("iota_m", [P, NB * HALF], bf16,
                             kind="ExternalInput")
    wcat_d = nc.dram_tensor("w_cat", [2 * D, D], bf16, kind="ExternalInput")
    out_d = nc.dram_tensor("out", [P, N_PAIR * D], f32, kind="ExternalOutput")

    AL = mybir.AluOpType
    ACT = mybir.ActivationFunctionType

    with tile.TileContext(nc) as tc:
        with (
            tc.tile_pool(name="meta", bufs=1) as meta,
            tc.tile_pool(name="gather", bufs=3) as gpool,
            tc.tile_pool(name="onehot", bufs=3) as opool,
            tc.tile_pool(name="const", bufs=1) as cpool,
            tc.tile_pool(name="fin", bufs=3) as fpool,
            tc.tile_pool(name="psum", bufs=4, space="PSUM") as psum,
            tc.tile_pool(name="psumT", bufs=2, space="PSUM") as psumT,
        ):
            meta_s = meta.tile([P, 5, C], bf16)
            rowoff_s = meta_s[:, 0, :]
            reprow_s = meta_s[:, 1, :]
            repc_s = meta_s[:, 2, :]
            sw_s = meta_s[:, 3, :]
            nsc_s = meta_s[:, 4, :]
            coefb = meta.tile([P, C], bf16)
            repsh_s = meta.tile([P, N_PAIR], f32)
            xselfb = meta.tile([P, N_PAIR, D], bf16)
            acc_all = meta.tile([P, N_PAIR, D + 1], f32)
            outs = meta.tile([P, N_PAIR, D], f32)
            wcat_s = cpool.tile([2 * D, D], bf16)
            ident = cpool.tile([P, P], bf16)
            iotaM = cpool.tile([P, NBG, HALF, ILV], bf16)
            # msg tiles are persistent (not pooled) so their deg-ones row is
            # written once in the prologue instead of every batch
            msg_bufs = [meta.tile([P, NBG, D + 1, ILV], bf16,
                                  name=f"msgbuf{k}")
                        for k in range(3)]
            # prepay the Q7 SWDGE ucode IRAM load (~6us) before batch 0's
            # cast-DMA needs it, overlapped with the prologue loads
            swdge_warm = cpool.tile([P, D], bf16)
            nc.gpsimd.dma_start(
                out=swdge_warm[:].rearrange("p d -> p d"),
                in_=xg_d[:, 0:D])
            # all prologue loads go on the scalar-engine HWDGE queue so the
            # sync queue carries nothing but the xg edge stream. The first
            # HEAD chunks of every per-chunk array load first (small, fixed-
            # cost DMAs) so the first batches start ~20us earlier; the bulk
            # tails follow.
            HEAD = 4 * NB
            meta_dv = meta_d[:].rearrange("p (k c) -> p k c", k=5)
            nc.scalar.dma_start(out=meta_s[:, :, :HEAD],
                                in_=meta_dv[:, :, :HEAD])
            nc.scalar.dma_start(
                out=iotaM[:].rearrange("p b j g -> p (b j g)"),
                in_=iotam_d[:])
            nc.scalar.dma_start(out=meta_s[:, :, HEAD:],
                                in_=meta_dv[:, :, HEAD:])

            make_identity(nc, ident[:])
            for mb in msg_bufs:
                nc.vector.memset(mb[:, :, D:D + 1, :], 1.0)

            # keep the PE clock gate (HAM) warm through the prologue
            warm_ps = psum.tile([P, D + 1], f32, tag="agg", name="warm_ps")
            for _ in range(40):
                nc.tensor.matmul(out=warm_ps[0:HALF, :],
                                 lhsT=ident[:, 0:HALF],
                                 rhs=ident[:, 0:D + 1],
                                 start=True, stop=True)

            # coef = sw * sigmoid(rep_row + rep_col) * ns_col. The head
            # slice is computed in the prologue; the tail pass is emitted
            # mid-loop (see below) so it does not block batches 0-2 in the
            # DVE instruction stream.
            def coef_pass(lo, hi):
                nc.vector.tensor_tensor(out=coefb[:, lo:hi],
                                        in0=reprow_s[:, lo:hi],
                                        in1=repc_s[:, lo:hi], op=AL.add)
                nc.scalar.activation(coefb[:, lo:hi], coefb[:, lo:hi],
                                     ACT.Sigmoid)
                nc.vector.tensor_tensor(out=coefb[:, lo:hi],
                                        in0=coefb[:, lo:hi],
                                        in1=sw_s[:, lo:hi], op=AL.mult)
                nc.vector.tensor_tensor(out=coefb[:, lo:hi],
                                        in0=coefb[:, lo:hi],
                                        in1=nsc_s[:, lo:hi], op=AL.mult)

            coef_pass(0, HEAD)

            # finalize-only inputs
            nc.scalar.dma_start(out=repsh_s[:], in_=repsh_d[:])
            nc.scalar.dma_start(out=xselfb[:].rearrange("p b d -> p (b d)"),
                                in_=xself_d[:])
            nc.scalar.dma_start(out=wcat_s[:], in_=wcat_d[:])

            def finalize_group(lo, n, drain=False):
                dg = fpool.tile([P, GRP], f32, tag="dg")
                nc.any.tensor_scalar_add(
                    out=dg[:, :n],
                    in0=acc_all[:, lo:lo + n, D:D + 1]
                        .rearrange("p b o -> p (b o)"),
                    scalar1=1e-6)
                nc.vector.reciprocal(out=dg[:, :n], in_=dg[:, :n])
                sr = fpool.tile([P, GRP], f32, tag="sr")
                nc.scalar.activation(sr[:, :n], repsh_s[:, lo:lo + n],
                                     ACT.Sigmoid)
                catg = fpool.tile([P, GRP, 2 * D], bf16, tag="catg")
                nc.vector.tensor_tensor(
                    out=catg[:, :n, 0:D], in0=acc_all[:, lo:lo + n, 0:D],
                    in1=dg[:, :n].rearrange("p (b o) -> p b o", o=1)
                        .to_broadcast([P, n, D]),
                    op=AL.mult)
                nc.vector.tensor_tensor(
                    out=catg[:, :n, D:2 * D], in0=xselfb[:, lo:lo + n, :],
                    in1=sr[:, :n].rearrange("p (b o) -> p b o", o=1)
                        .to_broadcast([P, n, D]),
                    op=AL.mult)
                for k in range(n):
                    pair = lo + k
                    ctp = psumT.tile([P, P], bf16, tag="ctp")
                    nc.tensor.transpose(out=ctp[:], in_=catg[:, k, :],
                                        identity=ident[:])
                    catT = fpool.tile([P, P], bf16, tag="catT")
                    if drain:
                        # in the drain the DVE is idle while ACT serializes
                        nc.vector.tensor_copy(out=catT[:], in_=ctp[:])
                    else:
                        nc.scalar.copy(catT[:], ctp[:])
                    out_ps = psumT.tile([P, D], f32, tag="out_ps")
                    nc.tensor.matmul(out=out_ps[:], lhsT=catT[:],
                                     rhs=wcat_s[:], start=True, stop=True)
                    nc.scalar.activation(outs[:, pair, :], out_ps[:],
                                         ACT.Lrelu, alpha=0.01)
                nc.sync.dma_start(
                    out=out_d[:, lo * D:(lo + n) * D],
                    in_=outs[:, lo:lo + n, :]
                        .rearrange("p b d -> p (b d)"))

            psum_cur = [None]
            pending = []   # finalize groups deferred to the next batch
            for bi, c0 in enumerate(range(0, C, NB)):
                xgb = gpool.tile([P, NBG, D, ILV], bf16, tag="xg")
                if bi < HEADB:
                    # head batches: plain HWDGE load of bf16-staged data, so
                    # the stream starts before the Q7 SWDGE ucode is loaded
                    nc.sync.dma_start(
                        out=xgb[:].rearrange("p b d g -> p (b d g)"),
                        in_=xgh_d[:, c0 * D:(c0 + NB) * D])
                else:
                    # fp8 in HBM, upconverted to bf16 in the SWDGE datapath
                    nc.gpsimd.dma_start(
                        out=xgb[:].rearrange("p b d g -> p (b d g)"),
                        in_=xg_d[:, c0 * D:(c0 + NB) * D])

                msg = msg_bufs[bi % 3]
                nc.vector.tensor_tensor(
                    out=msg[:, :, 0:D, :], in0=xgb[:],
                    in1=coefb[:, c0:c0 + NB]
                        .rearrange("p (b o g) -> p b o g", o=1, g=ILV)
                        .to_broadcast([P, NBG, D, ILV]),
                    op=AL.mult)

                oh = opool.tile([P, NBG, HALF, ILV], bf16, tag="oh")
                nc.vector.tensor_tensor(
                    out=oh[:],
                    in0=rowoff_s[:, c0:c0 + NB]
                        .rearrange("p (b o g) -> p b o g", o=1, g=ILV)
                        .to_broadcast([P, NBG, HALF, ILV]),
                    in1=iotaM[:],
                    op=AL.is_equal)

                if bi == 2:
                    # coef for batches 4+, emitted once batches 0-2 are in
                    # flight (their ops precede it in the DVE stream)
                    coef_pass(HEAD, C)

                # emit deferred finalize groups AFTER this batch's DVE prep:
                # their DVE/PE ops depend on earlier batches' matmuls, so
                # emitting them first would stall the DVE stream and starve
                # the PE of the next batch's one-hot/msg
                for lo, n in pending:
                    finalize_group(lo, n)
                pending = []

                for i in range(NB):
                    hb, is_start, is_stop = chunk_meta[c0 + i]
                    q = hb % QPP
                    if is_start and q == 0:
                        psum_cur[0] = psum.tile([P, D + 1], f32, tag="agg",
                                                name="agg_ps")
                    ps = psum_cur[0]
                    nc.tensor.matmul(
                        out=ps[q * HALF:(q + 1) * HALF, :],
                        lhsT=oh[:, i // ILV, :, i % ILV],
                        rhs=msg[:, i // ILV, 0:D + 1, i % ILV],
                        start=is_start, stop=is_stop,
                        tile_position=(0, q * HALF))
                    if is_stop and q == QPP - 1:
                        pair = hb // QPP
                        if c0 >= C - 2 * NB:
                            # drain region: ACT is the serializer, DVE idles
                            nc.vector.tensor_copy(out=acc_all[:, pair, :],
                                                  in_=ps[:])
                        else:
                            nc.scalar.copy(acc_all[:, pair, :], ps[:])
                        if pair + 1 in GROUP_ENDS:
                            # near the stream's end there is no later batch
                            # prep to protect; finalize eagerly to shorten
                            # the drain
                            if c0 >= C - 3 * NB:
                                finalize_group(*GROUP_ENDS[pair + 1],
                                               drain=True)
                            else:
                                pending.append(GROUP_ENDS[pair + 1])
            for lo, n in pending:
                finalize_group(lo, n, drain=True)

    nc.compile()
    return nc


def _preprocess(x, edge_index, sim_weight, rep, node_signal):
    """Host-side layout: group edges into (core, 64-row half-block) runs,
    pad to uniform chunk-aligned capacities, stage per-edge per-slot arrays
    (including the x[col] rows) in stream order."""
    import ml_dtypes

    bf = ml_dtypes.bfloat16
    row = np.ascontiguousarray(edge_index[0]).astype(np.int64)
    col = np.ascontiguousarray(edge_index[1]).astype(np.int64)
    sw = np.ascontiguousarray(sim_weight).astype(np.float32)
    rep_f = np.ascontiguousarray(rep).astype(np.float32)
    ns_f = np.ascontiguousarray(node_signal).astype(np.float32)
    x_f = np.ascontiguousarray(x).astype(np.float32)
    E = row.shape[0]

    core = row // N_LOC
    lrow = row - core * N_LOC

    # Balanced assignment of local rows to (half-block, offset): deal the
    # degree-sorted rows serpentine-wise across the 196 half-blocks (which
    # nearly equalizes per-run edge counts), then greedily swap rows between
    # bins until every bin is <= 1024 edges. hcap then rounds to 1024 for
    # almost every run instead of 1152 (~10% less chunk padding). The row
    # permutation is inverted when unsharding the output.
    n_real = (N_LOC + HALF - 1) // HALF
    CAP1 = 8 * P      # 1024
    deg_local = np.zeros((N_CORES, N_LOC), dtype=np.int64)
    np.add.at(deg_local, (core, lrow), 1)
    hb_map = np.empty((N_CORES, N_LOC), dtype=np.int64)
    off_map = np.empty((N_CORES, N_LOC), dtype=np.int64)
    for c in range(N_CORES):
        degs = deg_local[c]
        order_r = np.argsort(-degs, kind="stable")
        bins = [[] for _ in range(n_real)]
        sums = np.zeros(n_real, dtype=np.int64)
        for rnd in range((N_LOC + n_real - 1) // n_real):
            chunk = order_r[rnd * n_real:(rnd + 1) * n_real]
            order_b = range(n_real) if rnd % 2 == 0 else \
                range(n_real - 1, -1, -1)
            for k, b in enumerate(order_b):
                if k < len(chunk):
                    bins[b].append(int(chunk[k]))
                    sums[b] += int(degs[chunk[k]])
        # repair: swap rows out of over-cap bins into emptier bins until
        # every bin fits under CAP1
        for _ in range(20000):
            b = int(np.argmax(sums))
            if sums[b] <= CAP1:
                break
            ts = [int(t) for t in np.argsort(sums)[:24] if int(t) != b]
            us = sorted(range(len(bins[b])),
                        key=lambda i: -degs[bins[b][i]])[:12]
            done = False
            for t in ts:
                headroom = int(CAP1 - sums[t])
                for u in us:
                    du = int(degs[bins[b][u]])
                    best, best_dv = None, -1
                    for i, rv in enumerate(bins[t]):
                        dv = int(degs[rv])
                        if dv < du and du - dv <= headroom and dv > best_dv:
                            best, best_dv = i, dv
                    if best is not None:
                        ru, rv = bins[b][u], bins[t][best]
                        bins[b][u], bins[t][best] = rv, ru
                        sums[b] += best_dv - du
                        sums[t] += du - best_dv
                        done = True
                        break
                if done:
                    break
            if not done:
                break
        for b in range(n_real):
            for n, r in enumerate(bins[b]):
                hb_map[c, r] = b
                off_map[c, r] = n
    slot_row = hb_map * HALF + off_map          # [N_CORES, N_LOC]

    hb = hb_map[core, lrow]
    off = off_map[core, lrow].astype(np.float32)

    counts = np.zeros((N_CORES, N_HB), dtype=np.int64)
    np.add.at(counts, (core, hb), 1)
    maxc = counts.max(axis=0)
    assert maxc[:n_real].min() > 0, "empty sub-block run not supported"
    # virtual trailing sub-blocks (rows beyond N_LOC) still get one chunk of
    # pure padding so every pair's PSUM partitions are written
    hcap = np.maximum((-(-maxc // P) * P), P).astype(np.int64)
    # pad the last run so C is a multiple of NB (uniform full batches)
    c_raw = int(hcap.sum()) // P
    hcap[-1] += (-c_raw % NB) * P

    run_start_l, _, C = _layout(hcap)
    assert C % NB == 0
    run_start = np.array(run_start_l, dtype=np.int64)
    total = C * P

    key = core * N_HB + hb
    order = np.argsort(key, kind="stable")
    gcounts = np.bincount(key, minlength=N_CORES * N_HB)
    group_start = np.zeros(N_CORES * N_HB + 1, dtype=np.int64)
    np.cumsum(gcounts, out=group_start[1:])
    rank = np.arange(E, dtype=np.int64) - group_start[key[order]]
    ko = key[order]
    core_o = ko // N_HB
    hb_o = ko % N_HB
    gidx = core_o * total + run_start[hb_o] + rank

    tot = N_CORES * total
    rowoff_p = np.full(tot, DUMMY_OFF, dtype=np.float32)
    sw_p = np.zeros(tot, dtype=np.float32)
    reprow_p = np.zeros(tot, dtype=np.float32)
    repc_p = np.zeros(tot, dtype=np.float32)
    nsc_p = np.zeros(tot, dtype=np.float32)
    rowoff_p[gidx] = off[order]
    sw_p[gidx] = sw[order]
    reprow_p[gidx] = rep_f[row[order]]
    repc_p[gidx] = rep_f[col[order]]
    nsc_p[gidx] = ns_f[col[order]]
    xg = np.zeros((tot, D), dtype=np.float32)
    xg[gidx] = x_f[col[order]]

    def per_core(a):
        return np.ascontiguousarray(
            a.reshape(N_CORES, C, P).transpose(0, 2, 1).astype(bf))

    rowoff_t = per_core(rowoff_p)
    sw_t = per_core(sw_p)
    reprow_t = per_core(reprow_p)
    repc_t = per_core(repc_p)
    nsc_t = per_core(nsc_p)

    # xg stream: per batch of NB chunks, [128, NBG, D, ILV] interleaved so
    # the per-chunk PE operand stride is ILV elements
    xg16 = xg.astype(ml_dtypes.float8_e4m3).reshape(N_CORES, C, P, D)
    xgd = np.empty((N_CORES, P, C * D), dtype=ml_dtypes.float8_e4m3)
    for c0 in range(0, C, NB):
        blk = xg16[:, c0:c0 + NB].reshape(N_CORES, NBG, ILV, P, D)
        blk = blk.transpose(0, 3, 1, 4, 2)     # [8, 128, NBG, D, ILV]
        xgd[:, :, c0 * D:(c0 + NB) * D] = blk.reshape(N_CORES, P, NB * D)

    rep_pad = np.zeros((N_CORES, N_PAIR * P), dtype=np.float32)
    xs_pad = np.zeros((N_CORES, N_PAIR * P, D), dtype=np.float32)
    for c in range(N_CORES):
        rep_pad[c, slot_row[c]] = rep_f[c * N_LOC:(c + 1) * N_LOC]
        xs_pad[c, slot_row[c]] = x_f[c * N_LOC:(c + 1) * N_LOC]
    rep_sh = np.ascontiguousarray(
        rep_pad.reshape(N_CORES, N_PAIR, P).transpose(0, 2, 1))
    x_selfT = np.ascontiguousarray(
        xs_pad.reshape(N_CORES, N_PAIR, P, D).transpose(0, 2, 1, 3)
        .reshape(N_CORES, P, N_PAIR * D).astype(bf))

    iota_m = np.ascontiguousarray(
        np.broadcast_to(
            np.arange(HALF, dtype=np.float32)[None, None, :, None],
            (P, NBG, HALF, ILV)).reshape(P, NB * HALF).astype(bf))

    xg_head = np.ascontiguousarray(xgd[:, :, :3 * NB * D].astype(bf))

    meta5 = np.ascontiguousarray(
        np.stack([rowoff_t, reprow_t, repc_t, sw_t, nsc_t], axis=2)
        .reshape(N_CORES, P, 5 * C))

    return (hcap, xgd, xg_head, meta5, rep_sh, x_selfT, iota_m, slot_row)


_compiled = {}


def _get_program(hcap):
    key = tuple(hcap.tolist())
    if key not in _compiled:
        _compiled[key] = _build_program(hcap)
    return _compiled[key]


def run(x, edge_index, sim_weight, rep, node_signal, W, W_self, trace=False):
    import ml_dtypes
    from concourse.bass_utils import run_bass_kernel_spmd

    (hcap, xgd, xg_head, meta5, rep_sh, x_selfT, iota_m,
     slot_row) = _preprocess(x, edge_index, sim_weight, rep, node_signal)
    w_cat = np.ascontiguousarray(
        np.concatenate([np.asarray(W, dtype=np.float32),
                        np.asarray(W_self, dtype=np.float32)],
                       axis=0).astype(ml_dtypes.bfloat16))
    nc = _get_program(hcap)
    in_maps = []
    for c in range(N_CORES):
        in_maps.append({
            "xg": xgd[c],
            "xg_head": xg_head[c],
            "meta5": meta5[c],
            "rep_sh": rep_sh[c],
            "x_selfT": x_selfT[c],
            "iota_m": iota_m,
            "w_cat": w_cat,
        })
    res = run_bass_kernel_spmd(nc, in_maps, core_ids=list(range(N_CORES)),
                               trace=trace)
    parts = []
    for c in range(N_CORES):
        o = res.results[c]["out"].reshape(P, N_PAIR, D).transpose(1, 0, 2)
        parts.append(o.reshape(N_PAIR * P, D)[slot_row[c]])
    out = np.concatenate(parts, axis=0)
    return out, res


def kernel(x, edge_index, sim_weight, rep, node_signal, W, W_self):
    out, _ = run(x, edge_index, sim_weight, rep, node_signal, W, W_self)
    return out


# revision 77
# speedup vs baseline: 1.1536x; 1.0051x over previous
"""BehaviorAwareGCNLayer on 8 Trainium2 NeuronCores.

Math (reference):
    hx  = x @ W
    out[r] = (1/deg[r]) * sum_{e: row[e]=r} sim_w[e]*sigmoid(rep[row]+rep[col])*ns[col] * hx[col]
    out += sigmoid(rep) * (x @ W_self);  leaky_relu(out, 0.01)

Device strategy (destination sharding, no collectives):
  - By linearity, W is applied AFTER aggregation: agg[r] = sum coef_e * x[col_e],
    out[r] = (agg[r]/deg[r]) @ W + sigmoid(rep_r)*(x_r @ W_self).
  - Host does LAYOUT only (grouping/padding/fancy-index staging, same as the
    per-edge rep[row]/rep[col]/ns[col] arrays): it also stages the per-edge
    x[col] rows into slot order, so the device reads fully sequential
    streams instead of per-row gathers (dma_gather descriptor generation on
    GPSIMD was the original bottleneck: 2.5ms of Q7 busy time).
  - Core c owns destination rows [c*12500, (c+1)*12500). Edges are grouped
    into chunk-aligned runs by (core, 64-row half-block); run capacities are
    uniform across cores (max, rounded to 128) -> single SPMD program.
  - Slot (chunk ci, partition p) holds one edge. Per-batch tensors are
    chunk-interleaved ([128, NB/ILV, d-or-j, ILV]) so every DVE op has
    contiguous innermost APs on all operands -> 2x_1P perf mode (broadcasts
    ride outer dims), while PE operand slices keep a small ILV*2-byte
    stride (64B+ strides halved the LDWEIGHTS/matmul cadence).
    Per batch of NB chunks:
      * SWDGE DMA streams staged fp8-e4m3 x[col] rows, upconverting to
        bf16 in the DMA datapath (halves the dominant HBM stream)
      * msg[e, :, 0:64, g] = coef * x_col (bf16), row 64 = 1 (for deg)
      * one-hot oh[e, :, j, g] = (row_off == j), j in [0, 64)
      * per chunk, one PE matmul accumulates into the owning pair's PSUM:
        psum[half*64 + j, 0:65] += sum_e oh[e, j] * msg[e, :]
  - coef = sw * sigmoid(rep_row + rep_col) * ns_col is precomputed for ALL
    chunks in 4 bulk instructions at program start.
  - Per 128-row pair (two half-block runs share one [128, 65] PSUM tile):
    one ACT copy drains PSUM into a resident accumulator; every 14 pairs a
    grouped finalize does bulk 1/(deg+eps), sigmoid(rep), cat assembly, then
    per pair: PE transpose + one matmul with [W; W_self], ACT leaky-relu
    into a resident output tile; one bulk DMA out at the end.
"""
import sys

if "/opt/trn_rl_repo" not in sys.path:
    sys.path.insert(0, "/opt/trn_rl_repo")

import numpy as np

P = 128
D = 64
HALF = 64                              # one-hot width / sub-block rows
QPP = P // HALF                        # sub-blocks per 128-row output block
N_NODES = 100000
N_CORES = 8
N_LOC = N_NODES // N_CORES             # 12500 destination rows per core
N_PAIR = (N_LOC + P - 1) // P          # 98 output blocks
N_HB = N_PAIR * QPP                    # sub-blocks incl. trailing virtual pad
LAST_VALID = N_LOC - (N_PAIR - 1) * P  # 84 valid rows in last block
NB = 64                                # chunks per batch
ILV = 4                                # chunk interleave: PE operand stride
NBG = NB // ILV                        #   becomes ILV*2 bytes (4B at ILV=2)
GRP = 14                               # pairs per grouped finalize
# group boundaries: 14-pair groups, tail split finer to shorten the drain
_BOUNDS = [0, 14, 28, 42, 56, 70, 84, 88, 91, 93, 95, 96, 97, 98]
GROUP_ENDS = {_BOUNDS[i + 1]: (_BOUNDS[i], _BOUNDS[i + 1] - _BOUNDS[i])
              for i in range(len(_BOUNDS) - 1)}
DUMMY_OFF = 1000.0                     # one-hot-killing row offset for pads


def _layout(hcap):
    """Chunk-aligned run layout from per-half-block capacities (hcap[hb] is
    a multiple of P edges, shared across cores)."""
    run_start = [0] * N_HB             # slot index where hb's run begins
    chunk_meta = []                    # per chunk: (hb, is_start, is_stop)
    pos = 0
    for hb in range(N_HB):
        run_start[hb] = pos
        nch = int(hcap[hb]) // P
        for k in range(nch):
            chunk_meta.append((hb, k == 0, k == nch - 1))
        pos += int(hcap[hb])
    return run_start, chunk_meta, pos // P


def _build_program(hcap):
    """Emit + compile the single-core SPMD program."""
    import concourse.bacc as bacc
    import concourse.mybir as mybir
    import concourse.tile as tile
    from concourse.masks import make_identity

    f32 = mybir.dt.float32
    bf16 = mybir.dt.bfloat16
    f8 = mybir.dt.float8e4

    _, chunk_meta, C = _layout(hcap)

    nc = bacc.Bacc("TRN2", target_bir_lowering=False, debug=False)

    HEADB = 3          # leading batches staged bf16 (HWDGE, no Q7 wait)
    xg_d = nc.dram_tensor("xg", [P, C * D], f8, kind="ExternalInput")
    xgh_d = nc.dram_tensor("xg_head", [P, HEADB * NB * D], bf16,
                           kind="ExternalInput")
    # packed per-chunk metadata: [rowoff, reprow, repc, sw, nsc] along dim 1
    meta_d = nc.dram_tensor("meta5", [P, 5 * C], bf16, kind="ExternalInput")
    repsh_d = nc.dram_tensor("rep_sh", [P, N_PAIR], f32, kind="ExternalInput")
    xself_d = nc.dram_tensor("x_selfT", [P, N_PAIR * D], bf16,
                             kind="ExternalInput")
    iotam_d = nc.dram_tensor("iota_m", [P, NB * HALF], bf16,
                             kind="ExternalInput")
    wcat_d = nc.dram_tensor("w_cat", [2 * D, D], bf16, kind="ExternalInput")
    out_d = nc.dram_tensor("out", [P, N_PAIR * D], f32, kind="ExternalOutput")

    AL = mybir.AluOpType
    ACT = mybir.ActivationFunctionType

    with tile.TileContext(nc) as tc:
        with (
            tc.tile_pool(name="meta", bufs=1) as meta,
            tc.tile_pool(name="gather", bufs=3) as gpool,
            tc.tile_pool(name="onehot", bufs=3) as opool,
            tc.tile_pool(name="const", bufs=1) as cpool,
            tc.tile_pool(name="fin", bufs=3) as fpool,
            tc.tile_pool(name="psum", bufs=4, space="PSUM") as psum,
            tc.tile_pool(name="psumT", bufs=2, space="PSUM") as psumT,
        ):
            meta_s = meta.tile([P, 5, C], bf16)
            rowoff_s = meta_s[:, 0, :]
            reprow_s = meta_s[:, 1, :]
            repc_s = meta_s[:, 2, :]
            sw_s = meta_s[:, 3, :]
            nsc_s = meta_s[:, 4, :]
            coefb = meta.tile([P, C], bf16)
            repsh_s = meta.tile([P, N_PAIR], f32)
            xselfb = meta.tile([P, N_PAIR, D], bf16)
            acc_all = meta.tile([P, N_PAIR, D + 1], f32)
            outs = meta.tile([P, N_PAIR, D], f32)
            wcat_s = cpool.tile([2 * D, D], bf16)
            ident = cpool.tile([P, P], bf16)
            iotaM = cpool.tile([P, NBG, HALF, ILV], bf16)
            # msg tiles are persistent (not pooled) so their deg-ones row is
            # written once in the prologue instead of every batch
            msg_bufs = [meta.tile([P, NBG, D + 1, ILV], bf16,
                                  name=f"msgbuf{k}")
                        for k in range(3)]
            # prepay the Q7 SWDGE ucode IRAM load (~6us) before batch 0's
            # cast-DMA needs it, overlapped with the prologue loads
            swdge_warm = cpool.tile([P, D], bf16)
            nc.gpsimd.dma_start(
                out=swdge_warm[:].rearrange("p d -> p d"),
                in_=xg_d[:, 0:D])
            # all prologue loads go on the scalar-engine HWDGE queue so the
            # sync queue carries nothing but the xg edge stream. The first
            # HEAD chunks of every per-chunk array load first (small, fixed-
            # cost DMAs) so the first batches start ~20us earlier; the bulk
            # tails follow.
            HEAD = 4 * NB
            meta_dv = meta_d[:].rearrange("p (k c) -> p k c", k=5)
            nc.scalar.dma_start(out=meta_s[:, :, :HEAD],
                                in_=meta_dv[:, :, :HEAD])
            nc.scalar.dma_start(
                out=iotaM[:].rearrange("p b j g -> p (b j g)"),
                in_=iotam_d[:])
            nc.scalar.dma_start(out=meta_s[:, :, HEAD:],
                                in_=meta_dv[:, :, HEAD:])

            make_identity(nc, ident[:])
            for mb in msg_bufs:
                nc.vector.memset(mb[:, :, D:D + 1, :], 1.0)

            # keep the PE clock gate (HAM) warm through the prologue
            warm_ps = psum.tile([P, D + 1], f32, tag="agg", name="warm_ps")
            for _ in range(40):
                nc.tensor.matmul(out=warm_ps[0:HALF, :],
                                 lhsT=ident[:, 0:HALF],
                                 rhs=ident[:, 0:D + 1],
                                 start=True, stop=True)

            # coef = sw * sigmoid(rep_row + rep_col) * ns_col. The head
            # slice is computed in the prologue; the tail pass is emitted
            # mid-loop (see below) so it does not block batches 0-2 in the
            # DVE instruction stream.
            def coef_pass(lo, hi):
                nc.vector.tensor_tensor(out=coefb[:, lo:hi],
                                        in0=reprow_s[:, lo:hi],
                                        in1=repc_s[:, lo:hi], op=AL.add)
                nc.scalar.activation(coefb[:, lo:hi], coefb[:, lo:hi],
                                     ACT.Sigmoid)
                nc.vector.tensor_tensor(out=coefb[:, lo:hi],
                                        in0=coefb[:, lo:hi],
                                        in1=sw_s[:, lo:hi], op=AL.mult)
                nc.vector.tensor_tensor(out=coefb[:, lo:hi],
                                        in0=coefb[:, lo:hi],
                                        in1=nsc_s[:, lo:hi], op=AL.mult)

            coef_pass(0, HEAD)

            # finalize-only inputs
            nc.scalar.dma_start(out=repsh_s[:], in_=repsh_d[:])
            nc.scalar.dma_start(out=xselfb[:].rearrange("p b d -> p (b d)"),
                                in_=xself_d[:])
            nc.scalar.dma_start(out=wcat_s[:], in_=wcat_d[:])

            def finalize_group(lo, n, drain=False):
                dg = fpool.tile([P, GRP], f32, tag="dg")
                nc.any.tensor_scalar_add(
                    out=dg[:, :n],
                    in0=acc_all[:, lo:lo + n, D:D + 1]
                        .rearrange("p b o -> p (b o)"),
                    scalar1=1e-6)
                nc.vector.reciprocal(out=dg[:, :n], in_=dg[:, :n])
                sr = fpool.tile([P, GRP], f32, tag="sr")
                nc.scalar.activation(sr[:, :n], repsh_s[:, lo:lo + n],
                                     ACT.Sigmoid)
                catg = fpool.tile([P, GRP, 2 * D], bf16, tag="catg")
                nc.vector.tensor_tensor(
                    out=catg[:, :n, 0:D], in0=acc_all[:, lo:lo + n, 0:D],
                    in1=dg[:, :n].rearrange("p (b o) -> p b o", o=1)
                        .to_broadcast([P, n, D]),
                    op=AL.mult)
                nc.vector.tensor_tensor(
                    out=catg[:, :n, D:2 * D], in0=xselfb[:, lo:lo + n, :],
                    in1=sr[:, :n].rearrange("p (b o) -> p b o", o=1)
                        .to_broadcast([P, n, D]),
                    op=AL.mult)
                for k in range(n):
                    pair = lo + k
                    ctp = psumT.tile([P, P], bf16, tag="ctp")
                    nc.tensor.transpose(out=ctp[:], in_=catg[:, k, :],
                                        identity=ident[:])
                    catT = fpool.tile([P, P], bf16, tag="catT")
                    if drain:
                        # in the drain the DVE is idle while ACT serializes
                        nc.vector.tensor_copy(out=catT[:], in_=ctp[:])
                    else:
                        nc.scalar.copy(catT[:], ctp[:])
                    out_ps = psumT.tile([P, D], f32, tag="out_ps")
                    nc.tensor.matmul(out=out_ps[:], lhsT=catT[:],
                                     rhs=wcat_s[:], start=True, stop=True)
                    nc.scalar.activation(outs[:, pair, :], out_ps[:],
                                         ACT.Lrelu, alpha=0.01)
                nc.sync.dma_start(
                    out=out_d[:, lo * D:(lo + n) * D],
                    in_=outs[:, lo:lo + n, :]
                        .rearrange("p b d -> p (b d)"))

            psum_cur = [None]
            pending = []   # finalize groups deferred to the next batch
            for bi, c0 in enumerate(range(0, C, NB)):
                xgb = gpool.tile([P, NBG, D, ILV], bf16, tag="xg")
                if bi < HEADB:
                    # head batches: plain HWDGE load of bf16-staged data, so
                    # the stream starts before the Q7 SWDGE ucode is loaded
                    nc.sync.dma_start(
                        out=xgb[:].rearrange("p b d g -> p (b d g)"),
                        in_=xgh_d[:, c0 * D:(c0 + NB) * D])
                else:
                    # fp8 in HBM, upconverted to bf16 in the SWDGE datapath
                    nc.gpsimd.dma_start(
                        out=xgb[:].rearrange("p b d g -> p (b d g)"),
                        in_=xg_d[:, c0 * D:(c0 + NB) * D])

                msg = msg_bufs[bi % 3]
                nc.vector.tensor_tensor(
                    out=msg[:, :, 0:D, :], in0=xgb[:],
                    in1=coefb[:, c0:c0 + NB]
                        .rearrange("p (b o g) -> p b o g", o=1, g=ILV)
                        .to_broadcast([P, NBG, D, ILV]),
                    op=AL.mult)

                oh = opool.tile([P, NBG, HALF, ILV], bf16, tag="oh")
                nc.vector.tensor_tensor(
                    out=oh[:],
                    in0=rowoff_s[:, c0:c0 + NB]
                        .rearrange("p (b o g) -> p b o g", o=1, g=ILV)
                        .to_broadcast([P, NBG, HALF, ILV]),
                    in1=iotaM[:],
                    op=AL.is_equal)

                if bi == 2:
                    # coef for batches 4+: deprioritized so the scheduler
                    # cannot place it ahead of the first batches' one-hot/
                    # msg ops in the DVE stream (deps still force it before
                    # batch 4's msg multiply reads coefb)
                    with tc.high_priority(offset=-1000000):
                        coef_pass(HEAD, C)

                # emit deferred finalize groups AFTER this batch's DVE prep:
                # their DVE/PE ops depend on earlier batches' matmuls, so
                # emitting them first would stall the DVE stream and starve
                # the PE of the next batch's one-hot/msg
                for lo, n in pending:
                    finalize_group(lo, n)
                pending = []

                for i in range(NB):
                    hb, is_start, is_stop = chunk_meta[c0 + i]
                    q = hb % QPP
                    if is_start and q == 0:
                        psum_cur[0] = psum.tile([P, D + 1], f32, tag="agg",
                                                name="agg_ps")
                    ps = psum_cur[0]
                    nc.tensor.matmul(
                        out=ps[q * HALF:(q + 1) * HALF, :],
                        lhsT=oh[:, i // ILV, :, i % ILV],
                        rhs=msg[:, i // ILV, 0:D + 1, i % ILV],
                        start=is_start, stop=is_stop,
                        tile_position=(0, q * HALF))
                    if is_stop and q == QPP - 1:
                        pair = hb // QPP
                        if c0 >= C - 2 * NB:
                            # drain region: ACT is the serializer, DVE idles
                            nc.vector.tensor_copy(out=acc_all[:, pair, :],
                                                  in_=ps[:])
                        else:
                            nc.scalar.copy(acc_all[:, pair, :], ps[:])
                        if pair + 1 in GROUP_ENDS:
                            # near the stream's end there is no later batch
                            # prep to protect; finalize eagerly to shorten
                            # the drain
                            if c0 >= C - 3 * NB:
                                finalize_group(*GROUP_ENDS[pair + 1],
                                               drain=True)
                            else:
                                pending.append(GROUP_ENDS[pair + 1])
            for lo, n in pending:
                finalize_group(lo, n, drain=True)

    nc.compile()
    return nc


def _preprocess(x, edge_index, sim_weight, rep, node_signal):
    """Host-side layout: group edges into (core, 64-row half-block) runs,
    pad to uniform chunk-aligned capacities, stage per-edge per-slot arrays
    (including the x[col] rows) in stream order."""
    import ml_dtypes

    bf = ml_dtypes.bfloat16
    row = np.ascontiguousarray(edge_index[0]).astype(np.int64)
    col = np.ascontiguousarray(edge_index[1]).astype(np.int64)
    sw = np.ascontiguousarray(sim_weight).astype(np.float32)
    rep_f = np.ascontiguousarray(rep).astype(np.float32)
    ns_f = np.ascontiguousarray(node_signal).astype(np.float32)
    x_f = np.ascontiguousarray(x).astype(np.float32)
    E = row.shape[0]

    core = row // N_LOC
    lrow = row - core * N_LOC

    # Balanced assignment of local rows to (half-block, offset): deal the
    # degree-sorted rows serpentine-wise across the 196 half-blocks (which
    # nearly equalizes per-run edge counts), then greedily swap rows between
    # bins until every bin is <= 1024 edges. hcap then rounds to 1024 for
    # almost every run instead of 1152 (~10% less chunk padding). The row
    # permutation is inverted when unsharding the output.
    n_real = (N_LOC + HALF - 1) // HALF
    CAP1 = 8 * P      # 1024
    deg_local = np.zeros((N_CORES, N_LOC), dtype=np.int64)
    np.add.at(deg_local, (core, lrow), 1)
    hb_map = np.empty((N_CORES, N_LOC), dtype=np.int64)
    off_map = np.empty((N_CORES, N_LOC), dtype=np.int64)
    for c in range(N_CORES):
        degs = deg_local[c]
        order_r = np.argsort(-degs, kind="stable")
        bins = [[] for _ in range(n_real)]
        sums = np.zeros(n_real, dtype=np.int64)
        for rnd in range((N_LOC + n_real - 1) // n_real):
            chunk = order_r[rnd * n_real:(rnd + 1) * n_real]
            order_b = range(n_real) if rnd % 2 == 0 else \
                range(n_real - 1, -1, -1)
            for k, b in enumerate(order_b):
                if k < len(chunk):
                    bins[b].append(int(chunk[k]))
                    sums[b] += int(degs[chunk[k]])
        # repair: swap rows out of over-cap bins into emptier bins until
        # every bin fits under CAP1
        for _ in range(20000):
            b = int(np.argmax(sums))
            if sums[b] <= CAP1:
                break
            ts = [int(t) for t in np.argsort(sums)[:24] if int(t) != b]
            us = sorted(range(len(bins[b])),
                        key=lambda i: -degs[bins[b][i]])[:12]
            done = False
            for t in ts:
                headroom = int(CAP1 - sums[t])
                for u in us:
                    du = int(degs[bins[b][u]])
                    best, best_dv = None, -1
                    for i, rv in enumerate(bins[t]):
                        dv = int(degs[rv])
                        if dv < du and du - dv <= headroom and dv > best_dv:
                            best, best_dv = i, dv
                    if best is not None:
                        ru, rv = bins[b][u], bins[t][best]
                        bins[b][u], bins[t][best] = rv, ru
                        sums[b] += best_dv - du
                        sums[t] += du - best_dv
                        done = True
                        break
                if done:
                    break
            if not done:
                break
        for b in range(n_real):
            for n, r in enumerate(bins[b]):
                hb_map[c, r] = b
                off_map[c, r] = n
    slot_row = hb_map * HALF + off_map          # [N_CORES, N_LOC]

    hb = hb_map[core, lrow]
    off = off_map[core, lrow].astype(np.float32)

    counts = np.zeros((N_CORES, N_HB), dtype=np.int64)
    np.add.at(counts, (core, hb), 1)
    maxc = counts.max(axis=0)
    assert maxc[:n_real].min() > 0, "empty sub-block run not supported"
    # virtual trailing sub-blocks (rows beyond N_LOC) still get one chunk of
    # pure padding so every pair's PSUM partitions are written
    hcap = np.maximum((-(-maxc // P) * P), P).astype(np.int64)
    # pad the last run so C is a multiple of NB (uniform full batches)
    c_raw = int(hcap.sum()) // P
    hcap[-1] += (-c_raw % NB) * P

    run_start_l, _, C = _layout(hcap)
    assert C % NB == 0
    run_start = np.array(run_start_l, dtype=np.int64)
    total = C * P

    key = core * N_HB + hb
    order = np.argsort(key, kind="stable")
    gcounts = np.bincount(key, minlength=N_CORES * N_HB)
    group_start = np.zeros(N_CORES * N_HB + 1, dtype=np.int64)
    np.cumsum(gcounts, out=group_start[1:])
    rank = np.arange(E, dtype=np.int64) - group_start[key[order]]
    ko = key[order]
    core_o = ko // N_HB
    hb_o = ko % N_HB
    gidx = core_o * total + run_start[hb_o] + rank

    tot = N_CORES * total
    rowoff_p = np.full(tot, DUMMY_OFF, dtype=np.float32)
    sw_p = np.zeros(tot, dtype=np.float32)
    reprow_p = np.zeros(tot, dtype=np.float32)
    repc_p = np.zeros(tot, dtype=np.float32)
    nsc_p = np.zeros(tot, dtype=np.float32)
    rowoff_p[gidx] = off[order]
    sw_p[gidx] = sw[order]
    reprow_p[gidx] = rep_f[row[order]]
    repc_p[gidx] = rep_f[col[order]]
    nsc_p[gidx] = ns_f[col[order]]
    xg = np.zeros((tot, D), dtype=np.float32)
    xg[gidx] = x_f[col[order]]

    def per_core(a):
        return np.ascontiguousarray(
            a.reshape(N_CORES, C, P).transpose(0, 2, 1).astype(bf))

    rowoff_t = per_core(rowoff_p)
    sw_t = per_core(sw_p)
    reprow_t = per_core(reprow_p)
    repc_t = per_core(repc_p)
    nsc_t = per_core(nsc_p)

    # xg stream: per batch of NB chunks, [128, NBG, D, ILV] interleaved so
    # the per-chunk PE operand stride is ILV elements
    xg16 = xg.astype(ml_dtypes.float8_e4m3).reshape(N_CORES, C, P, D)
    xgd = np.empty((N_CORES, P, C * D), dtype=ml_dtypes.float8_e4m3)
    for c0 in range(0, C, NB):
        blk = xg16[:, c0:c0 + NB].reshape(N_CORES, NBG, ILV, P, D)
        blk = blk.transpose(0, 3, 1, 4, 2)     # [8, 128, NBG, D, ILV]
        xgd[:, :, c0 * D:(c0 + NB) * D] = blk.reshape(N_CORES, P, NB * D)

    rep_pad = np.zeros((N_CORES, N_PAIR * P), dtype=np.float32)
    xs_pad = np.zeros((N_CORES, N_PAIR * P, D), dtype=np.float32)
    for c in range(N_CORES):
        rep_pad[c, slot_row[c]] = rep_f[c * N_LOC:(c + 1) * N_LOC]
        xs_pad[c, slot_row[c]] = x_f[c * N_LOC:(c + 1) * N_LOC]
    rep_sh = np.ascontiguousarray(
        rep_pad.reshape(N_CORES, N_PAIR, P).transpose(0, 2, 1))
    x_selfT = np.ascontiguousarray(
        xs_pad.reshape(N_CORES, N_PAIR, P, D).transpose(0, 2, 1, 3)
        .reshape(N_CORES, P, N_PAIR * D).astype(bf))

    iota_m = np.ascontiguousarray(
        np.broadcast_to(
            np.arange(HALF, dtype=np.float32)[None, None, :, None],
            (P, NBG, HALF, ILV)).reshape(P, NB * HALF).astype(bf))

    xg_head = np.ascontiguousarray(xgd[:, :, :3 * NB * D].astype(bf))

    meta5 = np.ascontiguousarray(
        np.stack([rowoff_t, reprow_t, repc_t, sw_t, nsc_t], axis=2)
        .reshape(N_CORES, P, 5 * C))

    return (hcap, xgd, xg_head, meta5, rep_sh, x_selfT, iota_m, slot_row)


_compiled = {}


def _get_program(hcap):
    key = tuple(hcap.tolist())
    if key not in _compiled:
        _compiled[key] = _build_program(hcap)
    return _compiled[key]


def run(x, edge_index, sim_weight, rep, node_signal, W, W_self, trace=False):
    import ml_dtypes
    from concourse.bass_utils import run_bass_kernel_spmd

    (hcap, xgd, xg_head, meta5, rep_sh, x_selfT, iota_m,
     slot_row) = _preprocess(x, edge_index, sim_weight, rep, node_signal)
    w_cat = np.ascontiguousarray(
        np.concatenate([np.asarray(W, dtype=np.float32),
                        np.asarray(W_self, dtype=np.float32)],
                       axis=0).astype(ml_dtypes.bfloat16))
    nc = _get_program(hcap)
    in_maps = []
    for c in range(N_CORES):
        in_maps.append({
            "xg": xgd[c],
            "xg_head": xg_head[c],
            "meta5": meta5[c],
            "rep_sh": rep_sh[c],
            "x_selfT": x_selfT[c],
            "iota_m": iota_m,
            "w_cat": w_cat,
        })
    res = run_bass_kernel_spmd(nc, in_maps, core_ids=list(range(N_CORES)),
                               trace=trace)
    parts = []
    for c in range(N_CORES):
        o = res.results[c]["out"].reshape(P, N_PAIR, D).transpose(1, 0, 2)
        parts.append(o.reshape(N_PAIR * P, D)[slot_row[c]])
    out = np.concatenate(parts, axis=0)
    return out, res


def kernel(x, edge_index, sim_weight, rep, node_signal, W, W_self):
    out, _ = run(x, edge_index, sim_weight, rep, node_signal, W, W_self)
    return out


# revision 78
# speedup vs baseline: 1.1555x; 1.0016x over previous
"""BehaviorAwareGCNLayer on 8 Trainium2 NeuronCores.

Math (reference):
    hx  = x @ W
    out[r] = (1/deg[r]) * sum_{e: row[e]=r} sim_w[e]*sigmoid(rep[row]+rep[col])*ns[col] * hx[col]
    out += sigmoid(rep) * (x @ W_self);  leaky_relu(out, 0.01)

Device strategy (destination sharding, no collectives):
  - By linearity, W is applied AFTER aggregation: agg[r] = sum coef_e * x[col_e],
    out[r] = (agg[r]/deg[r]) @ W + sigmoid(rep_r)*(x_r @ W_self).
  - Host does LAYOUT only (grouping/padding/fancy-index staging, same as the
    per-edge rep[row]/rep[col]/ns[col] arrays): it also stages the per-edge
    x[col] rows into slot order, so the device reads fully sequential
    streams instead of per-row gathers (dma_gather descriptor generation on
    GPSIMD was the original bottleneck: 2.5ms of Q7 busy time).
  - Core c owns destination rows [c*12500, (c+1)*12500). Edges are grouped
    into chunk-aligned runs by (core, 64-row half-block); run capacities are
    uniform across cores (max, rounded to 128) -> single SPMD program.
  - Slot (chunk ci, partition p) holds one edge. Per-batch tensors are
    chunk-interleaved ([128, NB/ILV, d-or-j, ILV]) so every DVE op has
    contiguous innermost APs on all operands -> 2x_1P perf mode (broadcasts
    ride outer dims), while PE operand slices keep a small ILV*2-byte
    stride (64B+ strides halved the LDWEIGHTS/matmul cadence).
    Per batch of NB chunks:
      * SWDGE DMA streams staged fp8-e4m3 x[col] rows, upconverting to
        bf16 in the DMA datapath (halves the dominant HBM stream)
      * msg[e, :, 0:64, g] = coef * x_col (bf16), row 64 = 1 (for deg)
      * one-hot oh[e, :, j, g] = (row_off == j), j in [0, 64)
      * per chunk, one PE matmul accumulates into the owning pair's PSUM:
        psum[half*64 + j, 0:65] += sum_e oh[e, j] * msg[e, :]
  - coef = sw * sigmoid(rep_row + rep_col) * ns_col is precomputed for ALL
    chunks in 4 bulk instructions at program start.
  - Per 128-row pair (two half-block runs share one [128, 65] PSUM tile):
    one ACT copy drains PSUM into a resident accumulator; every 14 pairs a
    grouped finalize does bulk 1/(deg+eps), sigmoid(rep), cat assembly, then
    per pair: PE transpose + one matmul with [W; W_self], ACT leaky-relu
    into a resident output tile; one bulk DMA out at the end.
"""
import sys

if "/opt/trn_rl_repo" not in sys.path:
    sys.path.insert(0, "/opt/trn_rl_repo")

import numpy as np

P = 128
D = 64
HALF = 64                              # one-hot width / sub-block rows
QPP = P // HALF                        # sub-blocks per 128-row output block
N_NODES = 100000
N_CORES = 8
N_LOC = N_NODES // N_CORES             # 12500 destination rows per core
N_PAIR = (N_LOC + P - 1) // P          # 98 output blocks
N_HB = N_PAIR * QPP                    # sub-blocks incl. trailing virtual pad
LAST_VALID = N_LOC - (N_PAIR - 1) * P  # 84 valid rows in last block
NB = 64                                # chunks per batch
ILV = 4                                # chunk interleave: PE operand stride
NBG = NB // ILV                        #   becomes ILV*2 bytes (4B at ILV=2)
GRP = 14                               # pairs per grouped finalize
# group boundaries: 14-pair groups, tail split finer to shorten the drain
_BOUNDS = [0, 14, 28, 42, 56, 70, 84, 88, 91, 93, 95, 96, 97, 98]
GROUP_ENDS = {_BOUNDS[i + 1]: (_BOUNDS[i], _BOUNDS[i + 1] - _BOUNDS[i])
              for i in range(len(_BOUNDS) - 1)}
DUMMY_OFF = 1000.0                     # one-hot-killing row offset for pads


def _layout(hcap):
    """Chunk-aligned run layout from per-half-block capacities (hcap[hb] is
    a multiple of P edges, shared across cores)."""
    run_start = [0] * N_HB             # slot index where hb's run begins
    chunk_meta = []                    # per chunk: (hb, is_start, is_stop)
    pos = 0
    for hb in range(N_HB):
        run_start[hb] = pos
        nch = int(hcap[hb]) // P
        for k in range(nch):
            chunk_meta.append((hb, k == 0, k == nch - 1))
        pos += int(hcap[hb])
    return run_start, chunk_meta, pos // P


def _build_program(hcap):
    """Emit + compile the single-core SPMD program."""
    import concourse.bacc as bacc
    import concourse.mybir as mybir
    import concourse.tile as tile
    from concourse.masks import make_identity

    f32 = mybir.dt.float32
    bf16 = mybir.dt.bfloat16
    f8 = mybir.dt.float8e4

    _, chunk_meta, C = _layout(hcap)

    nc = bacc.Bacc("TRN2", target_bir_lowering=False, debug=False)

    HEADB = 3          # leading batches staged bf16 (HWDGE, no Q7 wait)
    xg_d = nc.dram_tensor("xg", [P, C * D], f8, kind="ExternalInput")
    xgh_d = nc.dram_tensor("xg_head", [P, HEADB * NB * D], bf16,
                           kind="ExternalInput")
    # packed per-chunk metadata: [rowoff, reprow, repc, sw, nsc] along dim 1
    meta_d = nc.dram_tensor("meta5", [P, 5 * C], bf16, kind="ExternalInput")
    repsh_d = nc.dram_tensor("rep_sh", [P, N_PAIR], f32, kind="ExternalInput")
    xself_d = nc.dram_tensor("x_selfT", [P, N_PAIR * D], bf16,
                             kind="ExternalInput")
    iotam_d = nc.dram_tensor("iota_m", [P, NB * HALF], bf16,
                             kind="ExternalInput")
    wcat_d = nc.dram_tensor("w_cat", [2 * D, D], bf16, kind="ExternalInput")
    out_d = nc.dram_tensor("out", [P, N_PAIR * D], f32, kind="ExternalOutput")

    AL = mybir.AluOpType
    ACT = mybir.ActivationFunctionType

    with tile.TileContext(nc) as tc:
        with (
            tc.tile_pool(name="meta", bufs=1) as meta,
            tc.tile_pool(name="gather", bufs=3) as gpool,
            tc.tile_pool(name="onehot", bufs=3) as opool,
            tc.tile_pool(name="const", bufs=1) as cpool,
            tc.tile_pool(name="fin", bufs=3) as fpool,
            tc.tile_pool(name="psum", bufs=4, space="PSUM") as psum,
            tc.tile_pool(name="psumT", bufs=2, space="PSUM") as psumT,
        ):
            meta_s = meta.tile([P, 5, C], bf16)
            rowoff_s = meta_s[:, 0, :]
            reprow_s = meta_s[:, 1, :]
            repc_s = meta_s[:, 2, :]
            sw_s = meta_s[:, 3, :]
            nsc_s = meta_s[:, 4, :]
            coefb = meta.tile([P, C], bf16)
            repsh_s = meta.tile([P, N_PAIR], f32)
            srep_all = meta.tile([P, N_PAIR], f32)
            xselfb = meta.tile([P, N_PAIR, D], bf16)
            cat_all = meta.tile([P, N_PAIR, 2 * D], bf16)
            acc_all = meta.tile([P, N_PAIR, D + 1], f32)
            outs = meta.tile([P, N_PAIR, D], f32)
            wcat_s = cpool.tile([2 * D, D], bf16)
            ident = cpool.tile([P, P], bf16)
            iotaM = cpool.tile([P, NBG, HALF, ILV], bf16)
            # msg tiles are persistent (not pooled) so their deg-ones row is
            # written once in the prologue instead of every batch
            msg_bufs = [meta.tile([P, NBG, D + 1, ILV], bf16,
                                  name=f"msgbuf{k}")
                        for k in range(3)]
            # prepay the Q7 SWDGE ucode IRAM load (~6us) before batch 0's
            # cast-DMA needs it, overlapped with the prologue loads
            swdge_warm = cpool.tile([P, D], bf16)
            nc.gpsimd.dma_start(
                out=swdge_warm[:].rearrange("p d -> p d"),
                in_=xg_d[:, 0:D])
            # all prologue loads go on the scalar-engine HWDGE queue so the
            # sync queue carries nothing but the xg edge stream. The first
            # HEAD chunks of every per-chunk array load first (small, fixed-
            # cost DMAs) so the first batches start ~20us earlier; the bulk
            # tails follow.
            HEAD = 4 * NB
            meta_dv = meta_d[:].rearrange("p (k c) -> p k c", k=5)
            nc.scalar.dma_start(out=meta_s[:, :, :HEAD],
                                in_=meta_dv[:, :, :HEAD])
            nc.scalar.dma_start(
                out=iotaM[:].rearrange("p b j g -> p (b j g)"),
                in_=iotam_d[:])
            nc.scalar.dma_start(out=meta_s[:, :, HEAD:],
                                in_=meta_dv[:, :, HEAD:])

            make_identity(nc, ident[:])
            for mb in msg_bufs:
                nc.vector.memset(mb[:, :, D:D + 1, :], 1.0)

            # keep the PE clock gate (HAM) warm through the prologue
            warm_ps = psum.tile([P, D + 1], f32, tag="agg", name="warm_ps")
            for _ in range(40):
                nc.tensor.matmul(out=warm_ps[0:HALF, :],
                                 lhsT=ident[:, 0:HALF],
                                 rhs=ident[:, 0:D + 1],
                                 start=True, stop=True)

            # coef = sw * sigmoid(rep_row + rep_col) * ns_col. The head
            # slice is computed in the prologue; the tail pass is emitted
            # mid-loop (see below) so it does not block batches 0-2 in the
            # DVE instruction stream.
            def coef_pass(lo, hi):
                nc.vector.tensor_tensor(out=coefb[:, lo:hi],
                                        in0=reprow_s[:, lo:hi],
                                        in1=repc_s[:, lo:hi], op=AL.add)
                nc.scalar.activation(coefb[:, lo:hi], coefb[:, lo:hi],
                                     ACT.Sigmoid)
                nc.vector.tensor_tensor(out=coefb[:, lo:hi],
                                        in0=coefb[:, lo:hi],
                                        in1=sw_s[:, lo:hi], op=AL.mult)
                nc.vector.tensor_tensor(out=coefb[:, lo:hi],
                                        in0=coefb[:, lo:hi],
                                        in1=nsc_s[:, lo:hi], op=AL.mult)

            coef_pass(0, HEAD)

            # finalize-only inputs
            nc.scalar.dma_start(out=repsh_s[:], in_=repsh_d[:])
            nc.scalar.dma_start(out=xselfb[:].rearrange("p b d -> p (b d)"),
                                in_=xself_d[:])
            nc.scalar.dma_start(out=wcat_s[:], in_=wcat_d[:])

            # the self-term half of cat does not depend on aggregation:
            # compute it once, DEPRIORITIZED so the scheduler slots it into
            # DVE idle time after its inputs land instead of blocking the
            # first batches at the head of the DVE stream
            with tc.high_priority(offset=-2000000):
                nc.scalar.activation(srep_all[:], repsh_s[:], ACT.Sigmoid)
                nc.vector.tensor_tensor(
                    out=cat_all[:, :, D:2 * D], in0=xselfb[:],
                    in1=srep_all[:].rearrange("p (b o) -> p b o", o=1)
                        .to_broadcast([P, N_PAIR, D]),
                    op=AL.mult)

            def finalize_group(lo, n, drain=False):
                dg = fpool.tile([P, GRP], f32, tag="dg")
                nc.any.tensor_scalar_add(
                    out=dg[:, :n],
                    in0=acc_all[:, lo:lo + n, D:D + 1]
                        .rearrange("p b o -> p (b o)"),
                    scalar1=1e-6)
                nc.vector.reciprocal(out=dg[:, :n], in_=dg[:, :n])
                nc.vector.tensor_tensor(
                    out=cat_all[:, lo:lo + n, 0:D],
                    in0=acc_all[:, lo:lo + n, 0:D],
                    in1=dg[:, :n].rearrange("p (b o) -> p b o", o=1)
                        .to_broadcast([P, n, D]),
                    op=AL.mult)
                for k in range(n):
                    pair = lo + k
                    ctp = psumT.tile([P, P], bf16, tag="ctp")
                    nc.tensor.transpose(out=ctp[:], in_=cat_all[:, pair, :],
                                        identity=ident[:])
                    catT = fpool.tile([P, P], bf16, tag="catT")
                    if drain:
                        # in the drain the DVE is idle while ACT serializes
                        nc.vector.tensor_copy(out=catT[:], in_=ctp[:])
                    else:
                        nc.scalar.copy(catT[:], ctp[:])
                    out_ps = psumT.tile([P, D], f32, tag="out_ps")
                    nc.tensor.matmul(out=out_ps[:], lhsT=catT[:],
                                     rhs=wcat_s[:], start=True, stop=True)
                    nc.scalar.activation(outs[:, pair, :], out_ps[:],
                                         ACT.Lrelu, alpha=0.01)
                nc.sync.dma_start(
                    out=out_d[:, lo * D:(lo + n) * D],
                    in_=outs[:, lo:lo + n, :]
                        .rearrange("p b d -> p (b d)"))

            psum_cur = [None]
            pending = []   # finalize groups deferred to the next batch
            for bi, c0 in enumerate(range(0, C, NB)):
                xgb = gpool.tile([P, NBG, D, ILV], bf16, tag="xg")
                if bi < HEADB:
                    # head batches: plain HWDGE load of bf16-staged data, so
                    # the stream starts before the Q7 SWDGE ucode is loaded
                    nc.sync.dma_start(
                        out=xgb[:].rearrange("p b d g -> p (b d g)"),
                        in_=xgh_d[:, c0 * D:(c0 + NB) * D])
                else:
                    # fp8 in HBM, upconverted to bf16 in the SWDGE datapath
                    nc.gpsimd.dma_start(
                        out=xgb[:].rearrange("p b d g -> p (b d g)"),
                        in_=xg_d[:, c0 * D:(c0 + NB) * D])

                msg = msg_bufs[bi % 3]
                nc.vector.tensor_tensor(
                    out=msg[:, :, 0:D, :], in0=xgb[:],
                    in1=coefb[:, c0:c0 + NB]
                        .rearrange("p (b o g) -> p b o g", o=1, g=ILV)
                        .to_broadcast([P, NBG, D, ILV]),
                    op=AL.mult)

                oh = opool.tile([P, NBG, HALF, ILV], bf16, tag="oh")
                nc.vector.tensor_tensor(
                    out=oh[:],
                    in0=rowoff_s[:, c0:c0 + NB]
                        .rearrange("p (b o g) -> p b o g", o=1, g=ILV)
                        .to_broadcast([P, NBG, HALF, ILV]),
                    in1=iotaM[:],
                    op=AL.is_equal)

                if bi == 2:
                    # coef for batches 4+: deprioritized so the scheduler
                    # cannot place it ahead of the first batches' one-hot/
                    # msg ops in the DVE stream (deps still force it before
                    # batch 4's msg multiply reads coefb)
                    with tc.high_priority(offset=-1000000):
                        coef_pass(HEAD, C)

                # emit deferred finalize groups AFTER this batch's DVE prep:
                # their DVE/PE ops depend on earlier batches' matmuls, so
                # emitting them first would stall the DVE stream and starve
                # the PE of the next batch's one-hot/msg
                for lo, n in pending:
                    finalize_group(lo, n)
                pending = []

                for i in range(NB):
                    hb, is_start, is_stop = chunk_meta[c0 + i]
                    q = hb % QPP
                    if is_start and q == 0:
                        psum_cur[0] = psum.tile([P, D + 1], f32, tag="agg",
                                                name="agg_ps")
                    ps = psum_cur[0]
                    nc.tensor.matmul(
                        out=ps[q * HALF:(q + 1) * HALF, :],
                        lhsT=oh[:, i // ILV, :, i % ILV],
                        rhs=msg[:, i // ILV, 0:D + 1, i % ILV],
                        start=is_start, stop=is_stop,
                        tile_position=(0, q * HALF))
                    if is_stop and q == QPP - 1:
                        pair = hb // QPP
                        if c0 >= C - 2 * NB:
                            # drain region: ACT is the serializer, DVE idles
                            nc.vector.tensor_copy(out=acc_all[:, pair, :],
                                                  in_=ps[:])
                        else:
                            nc.scalar.copy(acc_all[:, pair, :], ps[:])
                        if pair + 1 in GROUP_ENDS:
                            # near the stream's end there is no later batch
                            # prep to protect; finalize eagerly to shorten
                            # the drain
                            if c0 >= C - 3 * NB:
                                finalize_group(*GROUP_ENDS[pair + 1],
                                               drain=True)
                            else:
                                pending.append(GROUP_ENDS[pair + 1])
            for lo, n in pending:
                finalize_group(lo, n, drain=True)

    nc.compile()
    return nc


def _preprocess(x, edge_index, sim_weight, rep, node_signal):
    """Host-side layout: group edges into (core, 64-row half-block) runs,
    pad to uniform chunk-aligned capacities, stage per-edge per-slot arrays
    (including the x[col] rows) in stream order."""
    import ml_dtypes

    bf = ml_dtypes.bfloat16
    row = np.ascontiguousarray(edge_index[0]).astype(np.int64)
    col = np.ascontiguousarray(edge_index[1]).astype(np.int64)
    sw = np.ascontiguousarray(sim_weight).astype(np.float32)
    rep_f = np.ascontiguousarray(rep).astype(np.float32)
    ns_f = np.ascontiguousarray(node_signal).astype(np.float32)
    x_f = np.ascontiguousarray(x).astype(np.float32)
    E = row.shape[0]

    core = row // N_LOC
    lrow = row - core * N_LOC

    # Balanced assignment of local rows to (half-block, offset): deal the
    # degree-sorted rows serpentine-wise across the 196 half-blocks (which
    # nearly equalizes per-run edge counts), then greedily swap rows between
    # bins until every bin is <= 1024 edges. hcap then rounds to 1024 for
    # almost every run instead of 1152 (~10% less chunk padding). The row
    # permutation is inverted when unsharding the output.
    n_real = (N_LOC + HALF - 1) // HALF
    CAP1 = 8 * P      # 1024
    deg_local = np.zeros((N_CORES, N_LOC), dtype=np.int64)
    np.add.at(deg_local, (core, lrow), 1)
    hb_map = np.empty((N_CORES, N_LOC), dtype=np.int64)
    off_map = np.empty((N_CORES, N_LOC), dtype=np.int64)
    for c in range(N_CORES):
        degs = deg_local[c]
        order_r = np.argsort(-degs, kind="stable")
        bins = [[] for _ in range(n_real)]
        sums = np.zeros(n_real, dtype=np.int64)
        for rnd in range((N_LOC + n_real - 1) // n_real):
            chunk = order_r[rnd * n_real:(rnd + 1) * n_real]
            order_b = range(n_real) if rnd % 2 == 0 else \
                range(n_real - 1, -1, -1)
            for k, b in enumerate(order_b):
                if k < len(chunk):
                    bins[b].append(int(chunk[k]))
                    sums[b] += int(degs[chunk[k]])
        # repair: swap rows out of over-cap bins into emptier bins until
        # every bin fits under CAP1
        for _ in range(20000):
            b = int(np.argmax(sums))
            if sums[b] <= CAP1:
                break
            ts = [int(t) for t in np.argsort(sums)[:24] if int(t) != b]
            us = sorted(range(len(bins[b])),
                        key=lambda i: -degs[bins[b][i]])[:12]
            done = False
            for t in ts:
                headroom = int(CAP1 - sums[t])
                for u in us:
                    du = int(degs[bins[b][u]])
                    best, best_dv = None, -1
                    for i, rv in enumerate(bins[t]):
                        dv = int(degs[rv])
                        if dv < du and du - dv <= headroom and dv > best_dv:
                            best, best_dv = i, dv
                    if best is not None:
                        ru, rv = bins[b][u], bins[t][best]
                        bins[b][u], bins[t][best] = rv, ru
                        sums[b] += best_dv - du
                        sums[t] += du - best_dv
                        done = True
                        break
                if done:
                    break
            if not done:
                break
        for b in range(n_real):
            for n, r in enumerate(bins[b]):
                hb_map[c, r] = b
                off_map[c, r] = n
    slot_row = hb_map * HALF + off_map          # [N_CORES, N_LOC]

    hb = hb_map[core, lrow]
    off = off_map[core, lrow].astype(np.float32)

    counts = np.zeros((N_CORES, N_HB), dtype=np.int64)
    np.add.at(counts, (core, hb), 1)
    maxc = counts.max(axis=0)
    assert maxc[:n_real].min() > 0, "empty sub-block run not supported"
    # virtual trailing sub-blocks (rows beyond N_LOC) still get one chunk of
    # pure padding so every pair's PSUM partitions are written
    hcap = np.maximum((-(-maxc // P) * P), P).astype(np.int64)
    # pad the last run so C is a multiple of NB (uniform full batches)
    c_raw = int(hcap.sum()) // P
    hcap[-1] += (-c_raw % NB) * P

    run_start_l, _, C = _layout(hcap)
    assert C % NB == 0
    run_start = np.array(run_start_l, dtype=np.int64)
    total = C * P

    key = core * N_HB + hb
    order = np.argsort(key, kind="stable")
    gcounts = np.bincount(key, minlength=N_CORES * N_HB)
    group_start = np.zeros(N_CORES * N_HB + 1, dtype=np.int64)
    np.cumsum(gcounts, out=group_start[1:])
    rank = np.arange(E, dtype=np.int64) - group_start[key[order]]
    ko = key[order]
    core_o = ko // N_HB
    hb_o = ko % N_HB
    gidx = core_o * total + run_start[hb_o] + rank

    tot = N_CORES * total
    rowoff_p = np.full(tot, DUMMY_OFF, dtype=np.float32)
    sw_p = np.zeros(tot, dtype=np.float32)
    reprow_p = np.zeros(tot, dtype=np.float32)
    repc_p = np.zeros(tot, dtype=np.float32)
    nsc_p = np.zeros(tot, dtype=np.float32)
    rowoff_p[gidx] = off[order]
    sw_p[gidx] = sw[order]
    reprow_p[gidx] = rep_f[row[order]]
    repc_p[gidx] = rep_f[col[order]]
    nsc_p[gidx] = ns_f[col[order]]
    xg = np.zeros((tot, D), dtype=np.float32)
    xg[gidx] = x_f[col[order]]

    def per_core(a):
        return np.ascontiguousarray(
            a.reshape(N_CORES, C, P).transpose(0, 2, 1).astype(bf))

    rowoff_t = per_core(rowoff_p)
    sw_t = per_core(sw_p)
    reprow_t = per_core(reprow_p)
    repc_t = per_core(repc_p)
    nsc_t = per_core(nsc_p)

    # xg stream: per batch of NB chunks, [128, NBG, D, ILV] interleaved so
    # the per-chunk PE operand stride is ILV elements
    xg16 = xg.astype(ml_dtypes.float8_e4m3).reshape(N_CORES, C, P, D)
    xgd = np.empty((N_CORES, P, C * D), dtype=ml_dtypes.float8_e4m3)
    for c0 in range(0, C, NB):
        blk = xg16[:, c0:c0 + NB].reshape(N_CORES, NBG, ILV, P, D)
        blk = blk.transpose(0, 3, 1, 4, 2)     # [8, 128, NBG, D, ILV]
        xgd[:, :, c0 * D:(c0 + NB) * D] = blk.reshape(N_CORES, P, NB * D)

    rep_pad = np.zeros((N_CORES, N_PAIR * P), dtype=np.float32)
    xs_pad = np.zeros((N_CORES, N_PAIR * P, D), dtype=np.float32)
    for c in range(N_CORES):
        rep_pad[c, slot_row[c]] = rep_f[c * N_LOC:(c + 1) * N_LOC]
        xs_pad[c, slot_row[c]] = x_f[c * N_LOC:(c + 1) * N_LOC]
    rep_sh = np.ascontiguousarray(
        rep_pad.reshape(N_CORES, N_PAIR, P).transpose(0, 2, 1))
    x_selfT = np.ascontiguousarray(
        xs_pad.reshape(N_CORES, N_PAIR, P, D).transpose(0, 2, 1, 3)
        .reshape(N_CORES, P, N_PAIR * D).astype(bf))

    iota_m = np.ascontiguousarray(
        np.broadcast_to(
            np.arange(HALF, dtype=np.float32)[None, None, :, None],
            (P, NBG, HALF, ILV)).reshape(P, NB * HALF).astype(bf))

    xg_head = np.ascontiguousarray(xgd[:, :, :3 * NB * D].astype(bf))

    meta5 = np.ascontiguousarray(
        np.stack([rowoff_t, reprow_t, repc_t, sw_t, nsc_t], axis=2)
        .reshape(N_CORES, P, 5 * C))

    return (hcap, xgd, xg_head, meta5, rep_sh, x_selfT, iota_m, slot_row)


_compiled = {}


def _get_program(hcap):
    key = tuple(hcap.tolist())
    if key not in _compiled:
        _compiled[key] = _build_program(hcap)
    return _compiled[key]


def run(x, edge_index, sim_weight, rep, node_signal, W, W_self, trace=False):
    import ml_dtypes
    from concourse.bass_utils import run_bass_kernel_spmd

    (hcap, xgd, xg_head, meta5, rep_sh, x_selfT, iota_m,
     slot_row) = _preprocess(x, edge_index, sim_weight, rep, node_signal)
    w_cat = np.ascontiguousarray(
        np.concatenate([np.asarray(W, dtype=np.float32),
                        np.asarray(W_self, dtype=np.float32)],
                       axis=0).astype(ml_dtypes.bfloat16))
    nc = _get_program(hcap)
    in_maps = []
    for c in range(N_CORES):
        in_maps.append({
            "xg": xgd[c],
            "xg_head": xg_head[c],
            "meta5": meta5[c],
            "rep_sh": rep_sh[c],
            "x_selfT": x_selfT[c],
            "iota_m": iota_m,
            "w_cat": w_cat,
        })
    res = run_bass_kernel_spmd(nc, in_maps, core_ids=list(range(N_CORES)),
                               trace=trace)
    parts = []
    for c in range(N_CORES):
        o = res.results[c]["out"].reshape(P, N_PAIR, D).transpose(1, 0, 2)
        parts.append(o.reshape(N_PAIR * P, D)[slot_row[c]])
    out = np.concatenate(parts, axis=0)
    return out, res


def kernel(x, edge_index, sim_weight, rep, node_signal, W, W_self):
    out, _ = run(x, edge_index, sim_weight, rep, node_signal, W, W_self)
    return out
